# revision 18
# baseline (speedup 1.0000x reference)
"""
Trainium2 Bass kernel for nn_Local_Attention (segment-softmax attention over
atoms grouped into tokens).

Algorithm notes (reference semantics):
  q = (q_x @ Wq + bq) / sqrt(C)            [N, H*C]
  k = kv_x @ Wk ; v = kv_x @ Wv            [N, H*C]
  s[i,h] = sum_c q[i,h,c] k[i,h,c] / sqrt(C)
  alpha  = softmax of s over atoms sharing a token (segment softmax)
  out[t] = sum_{i in t} alpha[i] * v[i]    (only rows t < NUM_TOKENS nonzero)
  result = (out * sigmoid(q_x @ Wg)) @ Wo + bo

Key simplifications used here:
  * Scores are tiny (|s| < ~0.1), so the segment-max subtraction is skipped:
    alpha = e / segsum(e), e = exp(s). Numerator and denominator are both
    segment *sums*, and the division happens at token level:
    out[t] = segsum(e*v)[t] / segsum(e)[t].
  * Rows >= NUM_TOKENS of the result equal bo (segment sum there is zero), so
    only the first NUM_TOKENS rows are computed on device.
  * Segment sums are computed as one-hot matmuls: the host sorts atoms by
    token, packs 128 tokens per "group" (LPT-balanced), pads each group's
    atom list to a fixed tile capacity, and each 128-atom tile contributes
    via a [atom, token-slot] one-hot built on device from per-atom slot ids.

Sharding: 128 groups of 128 tokens each; 16 groups per core on 8 cores.
Projection weights are replicated.
"""

import math
import os
import sys

import numpy as np

sys.path.insert(0, "/opt/trn_rl_repo")

import ml_dtypes

BF16 = ml_dtypes.bfloat16

N = 262144
C_Q = 128
C_KV = 128
H = 4
C = 32
HC = H * C  # 128
NUM_TOKENS = 16384
NCORES = 8
GROUPS = 128          # token groups overall
TPG = 128             # tokens per group
GPC = GROUPS // NCORES  # groups per core = 16
INV_SQRT_C = 1.0 / math.sqrt(C)
PAD_SLOT = 255        # slot id for padding atoms (matches no token slot)

_BUILD_CACHE = {}
LAST_RESULTS = None  # stash of the last BassKernelResults for test harness


def _host_shard(atom_to_token_idx):
    """Assign tokens to 128 LPT-balanced groups of 128 tokens, sort atoms by
    (group, token), and compute the padded layout.

    Returns dict with permutation, destination indices, per-atom slot ids,
    token grid, and cap_tiles."""
    idx = np.asarray(atom_to_token_idx).astype(np.int64)
    counts = np.bincount(idx, minlength=NUM_TOKENS)

    # snake-deal tokens (sorted by size desc) into GROUPS groups
    order_tok = np.argsort(-counts, kind="stable")
    rounds = order_tok.reshape(NUM_TOKENS // GROUPS, GROUPS).copy()
    rounds[1::2] = rounds[1::2, ::-1]
    grp_of_tok = np.empty(NUM_TOKENS, np.int64)
    slot_of_tok = np.empty(NUM_TOKENS, np.int64)
    grp_of_tok[rounds] = np.broadcast_to(
        np.arange(GROUPS)[None, :], rounds.shape
    )
    slot_of_tok[rounds] = np.broadcast_to(
        np.arange(rounds.shape[0])[:, None], rounds.shape
    )
    # token id at (group, slot)
    tok_grid = np.empty((GROUPS, TPG), np.int64)
    tok_grid[grp_of_tok, slot_of_tok] = np.arange(NUM_TOKENS)

    loads = counts[tok_grid].sum(axis=1)  # atoms per group
    cap_tiles = max(1, int(math.ceil(loads.max() / 128.0)))
    cap_atoms = cap_tiles * 128

    # atoms sorted by (group, token id)
    key = grp_of_tok[idx] * NUM_TOKENS + idx
    perm = np.argsort(key, kind="stable")
    gidx = grp_of_tok[idx[perm]]           # nondecreasing group per atom
    group_start = np.searchsorted(gidx, np.arange(GROUPS))
    rank = np.arange(N) - group_start[gidx]
    dest = gidx * cap_atoms + rank         # position in padded atom array
    slots = slot_of_tok[idx[perm]]         # token slot of each (permuted) atom

    return dict(
        perm=perm,
        dest=dest,
        slots=slots,
        tok_grid=tok_grid,
        counts=counts,
        cap_tiles=cap_tiles,
        cap_atoms=cap_atoms,
    )


def _build_nc(cap_tiles, debug_dumps=False):
    """Build + schedule the SPMD Bass program for a given per-group tile
    capacity. Cached per cap_tiles."""
    key = (cap_tiles, debug_dumps)
    if key in _BUILD_CACHE:
        return _BUILD_CACHE[key]

    import concourse.bass as bass
    import concourse.tile as tile
    from concourse import bacc, mybir

    dt = mybir.dt
    AOT = mybir.AluOpType
    AFT = mybir.ActivationFunctionType

    cap_atoms = cap_tiles * 128
    atoms_pc = GPC * cap_atoms         # padded atoms per core
    tiles_pc = GPC * cap_tiles

    nc = bacc.Bacc(
        "TRN2", target_bir_lowering=False, debug=False, num_devices=NCORES
    )

    qxT_d = nc.dram_tensor("qxT", [128, atoms_pc], dt.bfloat16, kind="ExternalInput")
    kvT_d = nc.dram_tensor("kvT", [128, atoms_pc], dt.bfloat16, kind="ExternalInput")
    L_d = nc.dram_tensor("L", [128, tiles_pc], dt.float32, kind="ExternalInput")
    qxoT_d = nc.dram_tensor("qxoT", [128, GPC * TPG], dt.bfloat16, kind="ExternalInput")
    wq_d = nc.dram_tensor("wq", [128, HC], dt.bfloat16, kind="ExternalInput")
    wk_d = nc.dram_tensor("wk", [128, HC], dt.bfloat16, kind="ExternalInput")
    wv_d = nc.dram_tensor("wv", [128, HC], dt.bfloat16, kind="ExternalInput")
    wg_d = nc.dram_tensor("wg", [128, HC], dt.bfloat16, kind="ExternalInput")
    wo_d = nc.dram_tensor("wo", [HC, C_Q], dt.bfloat16, kind="ExternalInput")
    ind_d = nc.dram_tensor("ind", [HC, H], dt.bfloat16, kind="ExternalInput")
    ident_d = nc.dram_tensor("ident", [128, 128], dt.bfloat16, kind="ExternalInput")
    iota_d = nc.dram_tensor("iota", [128, 512], dt.float32, kind="ExternalInput")
    bq_d = nc.dram_tensor("bqv", [128, 1], dt.float32, kind="ExternalInput")
    out_d = nc.dram_tensor("out", [GPC * TPG, C_Q], dt.float32, kind="ExternalOutput")
    dbg = {}
    if debug_dumps:
        for nm, shp, dty in [
            ("dbg_q2", [128, 512], dt.bfloat16),
            ("dbg_qk", [128, 512], dt.bfloat16),
            ("dbg_e16", [128, 16], dt.bfloat16),
            ("dbg_a16", [128, 512], dt.bfloat16),
            ("dbg_w16", [128, 512], dt.bfloat16),
            ("dbg_grp", [128, 132], dt.float32),
            ("dbg_y0", [128, 128], dt.bfloat16),
            ("dbg_gate", [128, 128], dt.bfloat16),
            ("dbg_yT", [128, 128], dt.bfloat16),
        ]:
            dbg[nm] = nc.dram_tensor(nm, shp, dty, kind="ExternalOutput")

    # batches of up to 4 tiles within each group
    batches = []
    b0 = 0
    while b0 < cap_tiles:
        B = min(4, cap_tiles - b0)
        batches.append((b0, B))
        b0 += B

    with tile.TileContext(nc) as tc:
        with (
            tc.tile_pool(name="const", bufs=1) as cpool,
            tc.tile_pool(name="inp", bufs=2) as inp,
            tc.tile_pool(name="sb", bufs=2) as sb,
            tc.tile_pool(name="outp", bufs=2) as outp,
            tc.tile_pool(name="pgrp", bufs=1, space=bass.MemorySpace.PSUM) as pgrp,
            tc.tile_pool(name="pden", bufs=1, space=bass.MemorySpace.PSUM) as pden,
            tc.tile_pool(name="pq", bufs=2, space=bass.MemorySpace.PSUM) as pq,
            tc.tile_pool(name="pk", bufs=2, space=bass.MemorySpace.PSUM) as pk,
            tc.tile_pool(name="pv", bufs=2, space=bass.MemorySpace.PSUM) as pv,
        ):
            wq_sb = cpool.tile_from(wq_d[:])
            wk_sb = cpool.tile_from(wk_d[:])
            wv_sb = cpool.tile_from(wv_d[:])
            wg_sb = cpool.tile_from(wg_d[:])
            wo_sb = cpool.tile_from(wo_d[:])
            ind_sb = cpool.tile_from(ind_d[:])
            ident_sb = cpool.tile_from(ident_d[:])
            iota_sb = cpool.tile_from(iota_d[:])
            bq_sb = cpool.tile_from(bq_d[:])
            L_sb = cpool.tile_from(L_d[:])
            qxoT_sb = cpool.tile_from(qxoT_d[:])

            for g in range(GPC):
                qx_g = inp.tile([128, cap_atoms], dt.bfloat16, tag="qx")
                nc.sync.dma_start(
                    qx_g[:], qxT_d[:, g * cap_atoms : (g + 1) * cap_atoms]
                )
                kv_g = inp.tile([128, cap_atoms], dt.bfloat16, tag="kv")
                nc.sync.dma_start(
                    kv_g[:], kvT_d[:, g * cap_atoms : (g + 1) * cap_atoms]
                )

                grp_ps = pgrp.tile([128, 128], dt.float32, tag="grp")
                den_ps = pden.tile([128, H], dt.float32, tag="den")

                for (b0, B) in batches:
                    A = B * 128
                    off = b0 * 128
                    # feature-major q, k for the score chain
                    q_ps = pq.tile([128, A], dt.float32, tag="qp")
                    nc.tensor.matmul(
                        q_ps[:], wq_sb[:], qx_g[:, off : off + A],
                        start=True, stop=True,
                    )
                    k_ps = pk.tile([128, A], dt.float32, tag="kp")
                    nc.tensor.matmul(
                        k_ps[:], wk_sb[:], kv_g[:, off : off + A],
                        start=True, stop=True,
                    )
                    # q2 = q + bq (per-partition bias) on ACT, PSUM -> SBUF;
                    # then qk = q2 * k on DVE (only one PSUM operand allowed)
                    q2 = sb.tile([128, A], dt.bfloat16, tag="q2")
                    nc.scalar.activation(
                        q2[:], q_ps[:], AFT.Identity, bias=bq_sb[:]
                    )
                    qk = sb.tile([128, A], dt.bfloat16, tag="qk")
                    nc.vector.tensor_tensor(qk[:], q2[:], k_ps[:], AOT.mult)
                    # s[atom, h] per tile via PE reduction over hc partitions
                    # (shares PSUM slots with q_ps, which is dead by now)
                    s_ps = pq.tile([128, 4 * B], dt.float32, tag="qp")
                    for b in range(B):
                        nc.tensor.matmul(
                            s_ps[:, 4 * b : 4 * b + 4],
                            qk[:, 128 * b : 128 * (b + 1)],
                            ind_sb[:],
                            start=True, stop=True,
                        )
                    # e = exp(s): fp32 copy for the numerator path, bf16 for
                    # the denominator segment matmul
                    e32 = sb.tile([128, 4 * B], dt.float32, tag="e32")
                    nc.scalar.activation(e32[:], s_ps[:], AFT.Exp)
                    e16 = sb.tile([128, 4 * B], dt.bfloat16, tag="e16")
                    nc.scalar.activation(e16[:], s_ps[:], AFT.Exp)
                    # atom-major v
                    v_ps = pv.tile([128, A], dt.float32, tag="vp")
                    for b in range(B):
                        nc.tensor.matmul(
                            v_ps[:, 128 * b : 128 * (b + 1)],
                            kv_g[:, off + 128 * b : off + 128 * (b + 1)],
                            wv_sb[:],
                            start=True, stop=True,
                        )
                    # w = e * v  (e broadcast over the 32 c-columns per head)
                    w16 = sb.tile([128, A], dt.bfloat16, tag="w16")
                    v_view = v_ps[:].rearrange("p (b h c) -> p b h c", b=B, h=H, c=C)
                    w_view = w16[:].rearrange("p (b h c) -> p b h c", b=B, h=H, c=C)
                    e_view = (
                        e32[:]
                        .rearrange("p (b h) -> p b h", b=B, h=H)
                        .unsqueeze(-1)
                        .broadcast_to((128, B, H, C))
                    )
                    nc.vector.tensor_tensor(w_view, v_view, e_view, AOT.mult)
                    # one-hot [atom, token-slot] per tile, on gpsimd
                    a16 = sb.tile([128, A], dt.bfloat16, tag="a16")
                    for b in range(B):
                        nc.gpsimd.tensor_scalar(
                            a16[:, 128 * b : 128 * (b + 1)],
                            iota_sb[:, 0:128],
                            L_sb[:, g * cap_tiles + b0 + b : g * cap_tiles + b0 + b + 1],
                            None,
                            op0=AOT.is_equal,
                        )
                    if debug_dumps and g == 0 and b0 == 0:
                        nc.sync.dma_start(dbg["dbg_q2"][:], q2[:])
                        nc.sync.dma_start(dbg["dbg_qk"][:], qk[:])
                        nc.sync.dma_start(dbg["dbg_e16"][:], e16[:])
                        nc.sync.dma_start(dbg["dbg_a16"][:], a16[:])
                        nc.sync.dma_start(dbg["dbg_w16"][:], w16[:])
                    # segment-sum matmuls accumulate into the group PSUM
                    for b in range(B):
                        t = b0 + b
                        first = t == 0
                        last = t == cap_tiles - 1
                        nc.tensor.matmul(
                            grp_ps[:, 0:128],
                            a16[:, 128 * b : 128 * (b + 1)],
                            w16[:, 128 * b : 128 * (b + 1)],
                            start=first, stop=last,
                        )
                        nc.tensor.matmul(
                            den_ps[:],
                            a16[:, 128 * b : 128 * (b + 1)],
                            e16[:, 4 * b : 4 * b + 4],
                            start=first, stop=last,
                        )

                # ---- group stage: normalize, gate, project out ----
                if debug_dumps and g == 0:
                    grp_cp = sb.tile([128, 132], dt.float32, tag="grpcp")
                    nc.vector.tensor_copy(grp_cp[:, 0:128], grp_ps[:])
                    nc.vector.tensor_copy(grp_cp[:, 128:132], den_ps[:])
                    nc.sync.dma_start(dbg["dbg_grp"][:], grp_cp[:])
                r32 = sb.tile([128, H], dt.float32, tag="r32")
                nc.vector.reciprocal(r32[:], den_ps[:])
                y0 = sb.tile([128, HC], dt.bfloat16, tag="y0")
                num_view = grp_ps[:].rearrange("p (h c) -> p h c", h=H, c=C)
                y0_view = y0[:].rearrange("p (h c) -> p h c", h=H, c=C)
                r_view = r32[:].unsqueeze(-1).broadcast_to((128, H, C))
                nc.vector.tensor_tensor(y0_view, num_view, r_view, AOT.mult)

                g_ps = pq.tile([128, HC], dt.float32, tag="qp")
                nc.tensor.matmul(
                    g_ps[:], qxoT_sb[:, g * TPG : (g + 1) * TPG], wg_sb[:],
                    start=True, stop=True,
                )
                gate = sb.tile([128, HC], dt.bfloat16, tag="gate")
                nc.scalar.activation(gate[:], g_ps[:], AFT.Sigmoid)
                y = sb.tile([128, HC], dt.bfloat16, tag="y")
                nc.vector.tensor_tensor(y[:], y0[:], gate[:], AOT.mult)
                yT_ps = pk.tile([128, 128], dt.bfloat16, tag="kp")
                nc.tensor.transpose(yT_ps[:], y[:], ident_sb[:])
                yT16 = sb.tile([128, 128], dt.bfloat16, tag="yT")
                nc.scalar.activation(yT16[:], yT_ps[:], AFT.Copy)
                f_ps = pv.tile([128, C_Q], dt.float32, tag="vp")
                nc.tensor.matmul(
                    f_ps[:], yT16[:], wo_sb[:], start=True, stop=True
                )
                if debug_dumps and g == 0:
                    nc.sync.dma_start(dbg["dbg_y0"][:], y0[:])
                    nc.sync.dma_start(dbg["dbg_gate"][:], gate[:])
                    nc.sync.dma_start(dbg["dbg_yT"][:], yT16[:])
                o32 = outp.tile([128, C_Q], dt.float32, tag="o")
                nc.vector.tensor_copy(o32[:], f_ps[:])
                nc.sync.dma_start(out_d[g * TPG : (g + 1) * TPG, :], o32[:])

    nc.compile()
    _BUILD_CACHE[key] = nc
    return nc


def _install_ntff_shim():
    """The agent image's `antenv` lacks `axon_hooks`; recreate it and install
    the ctypes NTFF profile hook the way trn_agent_boot would."""
    import types

    import antenv

    if "antenv.axon_hooks" in sys.modules:
        return
    mod = types.ModuleType("antenv.axon_hooks")
    holder = [None]
    mod.set_axon_ntff_profile_hook = lambda h: holder.__setitem__(0, h)
    mod.get_axon_ntff_profile_hook = lambda: holder[0]
    sys.modules["antenv.axon_hooks"] = mod
    antenv.axon_hooks = mod
    try:
        sys.path.insert(0, "/root/.axon_site")
        from trn_agent_boot.trn_boot import _ntff_profile_via_ctypes

        hook = _ntff_profile_via_ctypes("/opt/axon/libaxon_pjrt.so")
        mod.set_axon_ntff_profile_hook(hook)
    except Exception as e:  # degrade to no tracing
        print(f"ntff shim install failed: {e}")


def kernel(q_x, kv_x, atom_to_token_idx, Wq, bq, Wk, Wv, Wg, Wo, bo):
    global LAST_RESULTS
    from concourse.bass_utils import run_bass_kernel_spmd

    q_x = np.asarray(q_x, np.float32)
    kv_x = np.asarray(kv_x, np.float32)
    Wq = np.asarray(Wq, np.float32)
    bq = np.asarray(bq, np.float32)
    Wk = np.asarray(Wk, np.float32)
    Wv = np.asarray(Wv, np.float32)
    Wg = np.asarray(Wg, np.float32)
    Wo = np.asarray(Wo, np.float32)
    bo = np.asarray(bo, np.float32)

    sh = _host_shard(atom_to_token_idx)
    cap_tiles = sh["cap_tiles"]
    cap_atoms = sh["cap_atoms"]
    perm, dest, slots = sh["perm"], sh["dest"], sh["slots"]
    tok_grid = sh["tok_grid"]

    # padded, permuted, bf16 inputs
    tot = GROUPS * cap_atoms
    Xq = np.zeros((tot, 128), BF16)
    Xq[dest] = q_x[perm].astype(BF16)
    Xkv = np.zeros((tot, 128), BF16)
    Xkv[dest] = kv_x[perm].astype(BF16)
    Lfull = np.full(tot, PAD_SLOT, np.float32)
    Lfull[dest] = slots.astype(np.float32)

    wq_h = (Wq * INV_SQRT_C).astype(BF16)
    wk_h = (Wk * INV_SQRT_C).astype(BF16)
    wv_h = Wv.astype(BF16)
    wg_h = Wg.astype(BF16)
    wo_h = Wo.astype(BF16)
    bq_h = (bq * INV_SQRT_C).astype(np.float32).reshape(128, 1)
    ind_h = np.zeros((HC, H), BF16)
    for h in range(H):
        ind_h[h * C : (h + 1) * C, h] = 1
    ident_h = np.eye(128, dtype=BF16)
    iota_h = np.broadcast_to(
        np.tile(np.arange(128, dtype=np.float32), 4), (128, 512)
    ).copy()

    apc = GPC * cap_atoms
    in_maps = []
    for c in range(NCORES):
        rows = slice(c * apc, (c + 1) * apc)
        qxT = np.ascontiguousarray(Xq[rows].T)
        kvT = np.ascontiguousarray(Xkv[rows].T)
        Lc = np.ascontiguousarray(
            Lfull[rows].reshape(GPC * cap_tiles, 128).T
        )
        tok_core = tok_grid[c * GPC : (c + 1) * GPC].reshape(GPC * TPG)
        qxoT = np.ascontiguousarray(q_x[tok_core].T.astype(BF16))
        in_maps.append(
            dict(
                qxT=qxT, kvT=kvT, L=Lc, qxoT=qxoT,
                wq=wq_h, wk=wk_h, wv=wv_h, wg=wg_h, wo=wo_h,
                ind=ind_h, ident=ident_h, iota=iota_h, bqv=bq_h,
            )
        )

    nc = _build_nc(cap_tiles, debug_dumps=os.environ.get("KERNEL_DEBUG_DUMPS", "0") == "1")
    trace = os.environ.get("KERNEL_TRACE", "0") == "1"
    if trace:
        _install_ntff_shim()
    res = run_bass_kernel_spmd(
        nc, in_maps, list(range(NCORES)), trace=trace,
        tmpdir=os.environ.get("KERNEL_TRACE_DIR") or None,
    )
    LAST_RESULTS = res

    out_full = np.broadcast_to(bo, (N, C_Q)).astype(np.float32).copy()
    for c in range(NCORES):
        tok_core = tok_grid[c * GPC : (c + 1) * GPC].reshape(GPC * TPG)
        out_full[tok_core] = res.results[c]["out"] + bo
    empty = np.where(sh["counts"] == 0)[0]
    if empty.size:
        out_full[empty] = bo
    return out_full


# revision 28
# speedup vs baseline: 2.3556x; 2.3556x over previous
"""
Trainium2 Bass kernel for nn_Local_Attention (segment-softmax attention over
atoms grouped into tokens).

Algorithm notes (reference semantics):
  q = (q_x @ Wq + bq) / sqrt(C)            [N, H*C]
  k = kv_x @ Wk ; v = kv_x @ Wv            [N, H*C]
  s[i,h] = sum_c q[i,h,c] k[i,h,c] / sqrt(C)
  alpha  = softmax of s over atoms sharing a token (segment softmax)
  out[t] = sum_{i in t} alpha[i] * v[i]    (only rows t < NUM_TOKENS nonzero)
  result = (out * sigmoid(q_x @ Wg)) @ Wo + bo

Key simplifications used here:
  * Scores are tiny (|s| < ~0.1), so the segment-max subtraction is skipped:
    alpha = e / segsum(e), e = exp(s). Numerator and denominator are both
    segment *sums*, and the division happens at token level:
    out[t] = segsum(e*v)[t] / segsum(e)[t].
  * Rows >= NUM_TOKENS of the result equal bo (segment sum there is zero), so
    only the first NUM_TOKENS rows are computed on device.
  * Segment sums are computed as one-hot matmuls: the host sorts atoms by
    token, packs 128 tokens per "group" (LPT-balanced), pads each group's
    atom list to a fixed tile capacity, and each 128-atom tile contributes
    via a [atom, token-slot] one-hot built on device from per-atom slot ids.

Sharding: 128 groups of 128 tokens each; 16 groups per core on 8 cores.
Projection weights are replicated.
"""

import math
import os
import sys

import numpy as np

sys.path.insert(0, "/opt/trn_rl_repo")

import ml_dtypes

BF16 = ml_dtypes.bfloat16

N = 262144
C_Q = 128
C_KV = 128
H = 4
C = 32
HC = H * C  # 128
NUM_TOKENS = 16384
NCORES = 8
GROUPS = 128          # token groups overall
TPG = 128             # tokens per group
GPC = GROUPS // NCORES  # groups per core = 16
INV_SQRT_C = 1.0 / math.sqrt(C)
PAD_SLOT = 255        # slot id for padding atoms (matches no token slot)

_BUILD_CACHE = {}
LAST_RESULTS = None  # stash of the last BassKernelResults for test harness


def _host_shard(atom_to_token_idx):
    """Assign tokens to 128 LPT-balanced groups of 128 tokens, sort atoms by
    (group, token), and compute the padded layout.

    Returns dict with permutation, destination indices, per-atom slot ids,
    token grid, and cap_tiles."""
    idx = np.asarray(atom_to_token_idx).astype(np.int64)
    counts = np.bincount(idx, minlength=NUM_TOKENS)

    # snake-deal tokens (sorted by size desc) into GROUPS groups
    order_tok = np.argsort(-counts, kind="stable")
    rounds = order_tok.reshape(NUM_TOKENS // GROUPS, GROUPS).copy()
    rounds[1::2] = rounds[1::2, ::-1]
    grp_of_tok = np.empty(NUM_TOKENS, np.int64)
    slot_of_tok = np.empty(NUM_TOKENS, np.int64)
    grp_of_tok[rounds] = np.broadcast_to(
        np.arange(GROUPS)[None, :], rounds.shape
    )
    slot_of_tok[rounds] = np.broadcast_to(
        np.arange(rounds.shape[0])[:, None], rounds.shape
    )
    # token id at (group, slot)
    tok_grid = np.empty((GROUPS, TPG), np.int64)
    tok_grid[grp_of_tok, slot_of_tok] = np.arange(NUM_TOKENS)

    loads = counts[tok_grid].sum(axis=1)  # atoms per group
    cap_tiles = max(1, int(math.ceil(loads.max() / 128.0)))
    cap_atoms = cap_tiles * 128

    # atoms sorted by (group, token id)
    key = grp_of_tok[idx] * NUM_TOKENS + idx
    perm = np.argsort(key, kind="stable")
    gidx = grp_of_tok[idx[perm]]           # nondecreasing group per atom
    group_start = np.searchsorted(gidx, np.arange(GROUPS))
    rank = np.arange(N) - group_start[gidx]
    dest = gidx * cap_atoms + rank         # position in padded atom array
    slots = slot_of_tok[idx[perm]]         # token slot of each (permuted) atom

    return dict(
        perm=perm,
        dest=dest,
        slots=slots,
        tok_grid=tok_grid,
        counts=counts,
        cap_tiles=cap_tiles,
        cap_atoms=cap_atoms,
    )


def _build_nc(cap_tiles, debug_dumps=False, has_bq=False):
    """Build + schedule the SPMD Bass program for a given per-group tile
    capacity. Cached per cap_tiles."""
    key = (cap_tiles, debug_dumps, has_bq)
    if key in _BUILD_CACHE:
        return _BUILD_CACHE[key]

    import concourse.bass as bass
    import concourse.tile as tile
    from concourse import bacc, mybir

    dt = mybir.dt
    AOT = mybir.AluOpType
    AFT = mybir.ActivationFunctionType

    cap_atoms = cap_tiles * 128
    atoms_pc = GPC * cap_atoms         # padded atoms per core
    tiles_pc = GPC * cap_tiles

    nc = bacc.Bacc(
        "TRN2", target_bir_lowering=False, debug=False, num_devices=NCORES
    )

    qxT_d = nc.dram_tensor("qxT", [128, atoms_pc], dt.bfloat16, kind="ExternalInput")
    kvT_d = nc.dram_tensor("kvT", [128, atoms_pc], dt.bfloat16, kind="ExternalInput")
    L_d = nc.dram_tensor("L", [128, tiles_pc], dt.float32, kind="ExternalInput")
    qxoT_d = nc.dram_tensor("qxoT", [128, GPC * TPG], dt.bfloat16, kind="ExternalInput")
    wq_d = nc.dram_tensor("wq", [128, HC], dt.bfloat16, kind="ExternalInput")
    wk_d = nc.dram_tensor("wk", [128, HC], dt.bfloat16, kind="ExternalInput")
    wv_d = nc.dram_tensor("wv", [128, HC], dt.bfloat16, kind="ExternalInput")
    wg_d = nc.dram_tensor("wg", [128, HC], dt.bfloat16, kind="ExternalInput")
    wo_d = nc.dram_tensor("wo", [HC, C_Q], dt.bfloat16, kind="ExternalInput")
    ind_d = nc.dram_tensor("ind", [HC, H], dt.bfloat16, kind="ExternalInput")
    ident_d = nc.dram_tensor("ident", [128, 128], dt.bfloat16, kind="ExternalInput")
    iota_d = nc.dram_tensor("iota", [128, 512], dt.float32, kind="ExternalInput")
    bq_d = nc.dram_tensor("bqv", [128, 1], dt.float32, kind="ExternalInput")
    out_d = nc.dram_tensor("out", [GPC * TPG, C_Q], dt.float32, kind="ExternalOutput")
    dbg = {}
    if debug_dumps:
        for nm, shp, dty in [
            ("dbg_q2", [128, 512], dt.bfloat16),
            ("dbg_qk", [128, 512], dt.bfloat16),
            ("dbg_a16", [128, 512], dt.bfloat16),
            ("dbg_grp", [128, 132], dt.float32),
            ("dbg_y0", [128, 128], dt.bfloat16),
            ("dbg_gate", [128, 128], dt.bfloat16),
            ("dbg_yT", [128, 128], dt.bfloat16),
        ]:
            dbg[nm] = nc.dram_tensor(nm, shp, dty, kind="ExternalOutput")

    # batches of up to 4 tiles within each group
    batches = []
    b0 = 0
    while b0 < cap_tiles:
        B = min(4, cap_tiles - b0)
        batches.append((b0, B))
        b0 += B

    with tile.TileContext(nc) as tc:
        with (
            tc.tile_pool(name="const", bufs=1) as cpool,
            tc.tile_pool(name="inp", bufs=2) as inp,
            tc.tile_pool(name="sb", bufs=2) as sb,
            tc.tile_pool(name="outp", bufs=2) as outp,
            tc.tile_pool(name="pgrp", bufs=2, space=bass.MemorySpace.PSUM) as pgrp,
            tc.tile_pool(name="pq", bufs=2, space=bass.MemorySpace.PSUM) as pq,
            tc.tile_pool(name="pk", bufs=2, space=bass.MemorySpace.PSUM) as pk,
            tc.tile_pool(name="pv", bufs=2, space=bass.MemorySpace.PSUM) as pv,
        ):
            wq_sb = cpool.tile_from(wq_d[:])
            wk_sb = cpool.tile_from(wk_d[:])
            wv_sb = cpool.tile_from(wv_d[:])
            wg_sb = cpool.tile_from(wg_d[:])
            wo_sb = cpool.tile_from(wo_d[:])
            ind_sb = cpool.tile_from(ind_d[:])
            ident_sb = cpool.tile_from(ident_d[:])
            iota_sb = cpool.tile_from(iota_d[:])
            bq_sb = cpool.tile_from(bq_d[:])
            L_sb = cpool.tile_from(L_d[:])
            qxoT_sb = cpool.tile_from(qxoT_d[:])

            for g in range(GPC):
                qx_g = inp.tile([128, cap_atoms], dt.bfloat16, tag="qx")
                nc.sync.dma_start(
                    qx_g[:], qxT_d[:, g * cap_atoms : (g + 1) * cap_atoms]
                )
                kv_g = inp.tile([128, cap_atoms], dt.bfloat16, tag="kv")
                nc.sync.dma_start(
                    kv_g[:], kvT_d[:, g * cap_atoms : (g + 1) * cap_atoms]
                )

                grp_ps = pgrp.tile([128, 132], dt.float32, tag="grp")

                for (b0, B) in batches:
                    A = B * 128
                    off = b0 * 128
                    # feature-major q, k for the score chain
                    q_ps = pq.tile([128, A], dt.float32, tag="qp")
                    nc.tensor.matmul(
                        q_ps[:], wq_sb[:], qx_g[:, off : off + A],
                        start=True, stop=True,
                    )
                    k_ps = pk.tile([128, A], dt.float32, tag="kp")
                    nc.tensor.matmul(
                        k_ps[:], wk_sb[:], kv_g[:, off : off + A],
                        start=True, stop=True,
                    )
                    # q2 = q + bq (per-partition bias) on ACT, PSUM -> SBUF;
                    # then qk = q2 * k on DVE (only one PSUM operand allowed)
                    q2 = sb.tile([128, A], dt.bfloat16, tag="q2")
                    if has_bq:
                        nc.scalar.activation(
                            q2[:], q_ps[:], AFT.Identity, bias=bq_sb[:]
                        )
                    else:
                        nc.scalar.activation(q2[:], q_ps[:], AFT.Copy)
                    qk = sb.tile([128, A], dt.bfloat16, tag="qk")
                    nc.vector.tensor_tensor(qk[:], q2[:], k_ps[:], AOT.mult)
                    # s[atom, h] per tile via PE reduction over hc partitions
                    # (shares PSUM slots with q_ps, which is dead by now)
                    s_ps = pq.tile([128, 4 * B], dt.float32, tag="qp")
                    for b in range(B):
                        nc.tensor.matmul(
                            s_ps[:, 4 * b : 4 * b + 4],
                            qk[:, 128 * b : 128 * (b + 1)],
                            ind_sb[:],
                            start=True, stop=True,
                        )
                    # e = exp(s) in fp32; bf16 copy lands in the fused [w|e]
                    # rhs tile for the segment matmul
                    e32 = sb.tile([128, 4 * B], dt.float32, tag="e32")
                    nc.scalar.activation(e32[:], s_ps[:], AFT.Exp)
                    # atom-major v
                    v_ps = pv.tile([128, A], dt.float32, tag="vp")
                    for b in range(B):
                        nc.tensor.matmul(
                            v_ps[:, 128 * b : 128 * (b + 1)],
                            kv_g[:, off + 128 * b : off + 128 * (b + 1)],
                            wv_sb[:],
                            start=True, stop=True,
                        )
                    # fused rhs tile: per tile 132 cols = [w (128) | e (4)]
                    we = sb.tile([128, B * 132], dt.bfloat16, tag="we")
                    we3 = we[:].rearrange("p (b f) -> p b f", b=B, f=132)
                    w_view = we3[:, :, 0:128].rearrange(
                        "p b (h c) -> p b h c", h=H, c=C
                    )
                    v_view = v_ps[:].rearrange("p (b h c) -> p b h c", b=B, h=H, c=C)
                    e_view = (
                        e32[:]
                        .rearrange("p (b h) -> p b h", b=B, h=H)
                        .unsqueeze(-1)
                        .broadcast_to((128, B, H, C))
                    )
                    nc.vector.tensor_tensor(w_view, v_view, e_view, AOT.mult)
                    nc.vector.tensor_copy(
                        we3[:, :, 128:132],
                        e32[:].rearrange("p (b h) -> p b h", b=B, h=H),
                    )
                    # one-hot [atom, token-slot], one batched DVE op
                    a16 = sb.tile([128, A], dt.bfloat16, tag="a16")
                    nc.vector.tensor_tensor(
                        a16[:].rearrange("p (b t) -> p b t", b=B, t=128),
                        iota_sb[:, 0:A].rearrange("p (b t) -> p b t", b=B, t=128),
                        L_sb[
                            :, g * cap_tiles + b0 : g * cap_tiles + b0 + B
                        ].unsqueeze(-1).broadcast_to((128, B, 128)),
                        AOT.is_equal,
                    )
                    if debug_dumps and g == 0 and b0 == 0:
                        nc.sync.dma_start(dbg["dbg_q2"][:], q2[:])
                        nc.sync.dma_start(dbg["dbg_qk"][:], qk[:])
                        nc.sync.dma_start(dbg["dbg_a16"][:], a16[:])
                    # one fused segment-sum matmul per tile: [num | den]
                    for b in range(B):
                        t = b0 + b
                        nc.tensor.matmul(
                            grp_ps[:],
                            a16[:, 128 * b : 128 * (b + 1)],
                            we[:, 132 * b : 132 * (b + 1)],
                            start=(t == 0), stop=(t == cap_tiles - 1),
                        )

                # ---- group stage: normalize, gate, project out ----
                if debug_dumps and g == 0:
                    grp_cp = sb.tile([128, 132], dt.float32, tag="grpcp")
                    nc.vector.tensor_copy(grp_cp[:], grp_ps[:])
                    nc.sync.dma_start(dbg["dbg_grp"][:], grp_cp[:])
                r32 = sb.tile([128, H], dt.float32, tag="r32")
                nc.vector.reciprocal(r32[:], grp_ps[:, 128:132])
                y0 = sb.tile([128, HC], dt.bfloat16, tag="y0")
                num_view = grp_ps[:, 0:128].rearrange("p (h c) -> p h c", h=H, c=C)
                y0_view = y0[:].rearrange("p (h c) -> p h c", h=H, c=C)
                r_view = r32[:].unsqueeze(-1).broadcast_to((128, H, C))
                nc.vector.tensor_tensor(y0_view, num_view, r_view, AOT.mult)

                g_ps = pq.tile([128, HC], dt.float32, tag="qp")
                nc.tensor.matmul(
                    g_ps[:], qxoT_sb[:, g * TPG : (g + 1) * TPG], wg_sb[:],
                    start=True, stop=True,
                )
                gate = sb.tile([128, HC], dt.bfloat16, tag="gate")
                nc.scalar.activation(gate[:], g_ps[:], AFT.Sigmoid)
                y = sb.tile([128, HC], dt.bfloat16, tag="y")
                nc.vector.tensor_tensor(y[:], y0[:], gate[:], AOT.mult)
                yT_ps = pk.tile([128, 128], dt.bfloat16, tag="kp")
                nc.tensor.transpose(yT_ps[:], y[:], ident_sb[:])
                yT16 = sb.tile([128, 128], dt.bfloat16, tag="yT")
                nc.scalar.activation(yT16[:], yT_ps[:], AFT.Copy)
                f_ps = pv.tile([128, C_Q], dt.float32, tag="vp")
                nc.tensor.matmul(
                    f_ps[:], yT16[:], wo_sb[:], start=True, stop=True
                )
                if debug_dumps and g == 0:
                    nc.sync.dma_start(dbg["dbg_y0"][:], y0[:])
                    nc.sync.dma_start(dbg["dbg_gate"][:], gate[:])
                    nc.sync.dma_start(dbg["dbg_yT"][:], yT16[:])
                o32 = outp.tile([128, C_Q], dt.float32, tag="o")
                nc.vector.tensor_copy(o32[:], f_ps[:])
                nc.sync.dma_start(out_d[g * TPG : (g + 1) * TPG, :], o32[:])

    nc.compile()
    _BUILD_CACHE[key] = nc
    return nc


def _install_ntff_shim():
    """The agent image's `antenv` lacks `axon_hooks`; recreate it and install
    the ctypes NTFF profile hook the way trn_agent_boot would."""
    import types

    import antenv

    if "antenv.axon_hooks" in sys.modules:
        return
    mod = types.ModuleType("antenv.axon_hooks")
    holder = [None]
    mod.set_axon_ntff_profile_hook = lambda h: holder.__setitem__(0, h)
    mod.get_axon_ntff_profile_hook = lambda: holder[0]
    sys.modules["antenv.axon_hooks"] = mod
    antenv.axon_hooks = mod
    try:
        sys.path.insert(0, "/root/.axon_site")
        from trn_agent_boot.trn_boot import _ntff_profile_via_ctypes

        hook = _ntff_profile_via_ctypes("/opt/axon/libaxon_pjrt.so")
        mod.set_axon_ntff_profile_hook(hook)
    except Exception as e:  # degrade to no tracing
        print(f"ntff shim install failed: {e}")


def kernel(q_x, kv_x, atom_to_token_idx, Wq, bq, Wk, Wv, Wg, Wo, bo):
    global LAST_RESULTS
    from concourse.bass_utils import run_bass_kernel_spmd

    q_x = np.asarray(q_x, np.float32)
    kv_x = np.asarray(kv_x, np.float32)
    Wq = np.asarray(Wq, np.float32)
    bq = np.asarray(bq, np.float32)
    Wk = np.asarray(Wk, np.float32)
    Wv = np.asarray(Wv, np.float32)
    Wg = np.asarray(Wg, np.float32)
    Wo = np.asarray(Wo, np.float32)
    bo = np.asarray(bo, np.float32)

    sh = _host_shard(atom_to_token_idx)
    cap_tiles = sh["cap_tiles"]
    cap_atoms = sh["cap_atoms"]
    perm, dest, slots = sh["perm"], sh["dest"], sh["slots"]
    tok_grid = sh["tok_grid"]

    # padded, permuted, bf16 inputs
    tot = GROUPS * cap_atoms
    Xq = np.zeros((tot, 128), BF16)
    Xq[dest] = q_x[perm].astype(BF16)
    Xkv = np.zeros((tot, 128), BF16)
    Xkv[dest] = kv_x[perm].astype(BF16)
    Lfull = np.full(tot, PAD_SLOT, np.float32)
    Lfull[dest] = slots.astype(np.float32)

    wq_h = (Wq * INV_SQRT_C).astype(BF16)
    wk_h = (Wk * INV_SQRT_C).astype(BF16)
    wv_h = Wv.astype(BF16)
    wg_h = Wg.astype(BF16)
    wo_h = Wo.astype(BF16)
    bq_h = (bq * INV_SQRT_C).astype(np.float32).reshape(128, 1)
    ind_h = np.zeros((HC, H), BF16)
    for h in range(H):
        ind_h[h * C : (h + 1) * C, h] = 1
    ident_h = np.eye(128, dtype=BF16)
    iota_h = np.broadcast_to(
        np.tile(np.arange(128, dtype=np.float32), 4), (128, 512)
    ).copy()

    apc = GPC * cap_atoms
    in_maps = []
    for c in range(NCORES):
        rows = slice(c * apc, (c + 1) * apc)
        qxT = np.ascontiguousarray(Xq[rows].T)
        kvT = np.ascontiguousarray(Xkv[rows].T)
        Lc = np.ascontiguousarray(
            Lfull[rows].reshape(GPC * cap_tiles, 128).T
        )
        tok_core = tok_grid[c * GPC : (c + 1) * GPC].reshape(GPC * TPG)
        qxoT = np.ascontiguousarray(q_x[tok_core].T.astype(BF16))
        in_maps.append(
            dict(
                qxT=qxT, kvT=kvT, L=Lc, qxoT=qxoT,
                wq=wq_h, wk=wk_h, wv=wv_h, wg=wg_h, wo=wo_h,
                ind=ind_h, ident=ident_h, iota=iota_h, bqv=bq_h,
            )
        )

    nc = _build_nc(
        cap_tiles,
        debug_dumps=os.environ.get("KERNEL_DEBUG_DUMPS", "0") == "1",
        has_bq=bool(np.any(bq != 0)),
    )
    trace = os.environ.get("KERNEL_TRACE", "0") == "1"
    if trace:
        _install_ntff_shim()
    res = run_bass_kernel_spmd(
        nc, in_maps, list(range(NCORES)), trace=trace,
        tmpdir=os.environ.get("KERNEL_TRACE_DIR") or None,
    )
    LAST_RESULTS = res

    out_full = np.broadcast_to(bo, (N, C_Q)).astype(np.float32).copy()
    for c in range(NCORES):
        tok_core = tok_grid[c * GPC : (c + 1) * GPC].reshape(GPC * TPG)
        out_full[tok_core] = res.results[c]["out"] + bo
    empty = np.where(sh["counts"] == 0)[0]
    if empty.size:
        out_full[empty] = bo
    return out_full


# revision 41
# speedup vs baseline: 2.3864x; 1.0131x over previous
"""
Trainium2 Bass kernel for nn_Local_Attention (segment-softmax attention over
atoms grouped into tokens).

Algorithm notes (reference semantics):
  q = (q_x @ Wq + bq) / sqrt(C)            [N, H*C]
  k = kv_x @ Wk ; v = kv_x @ Wv            [N, H*C]
  s[i,h] = sum_c q[i,h,c] k[i,h,c] / sqrt(C)
  alpha  = softmax of s over atoms sharing a token (segment softmax)
  out[t] = sum_{i in t} alpha[i] * v[i]    (only rows t < NUM_TOKENS nonzero)
  result = (out * sigmoid(q_x @ Wg)) @ Wo + bo

Key simplifications used here:
  * Scores are tiny (|s| < ~0.1), so the segment-max subtraction is skipped:
    alpha = e / segsum(e), e = exp(s). Numerator and denominator are both
    segment *sums*, and the division happens at token level:
    out[t] = segsum(e*v)[t] / segsum(e)[t].
  * Rows >= NUM_TOKENS of the result equal bo (segment sum there is zero), so
    only the first NUM_TOKENS rows are computed on device.
  * Segment sums are computed as one-hot matmuls: the host sorts atoms by
    token, packs 128 tokens per "group" (LPT-balanced), pads each group's
    atom list to a fixed tile capacity, and each 128-atom tile contributes
    via a [atom, token-slot] one-hot built on device from per-atom slot ids.

Sharding: 128 groups of 128 tokens each; 16 groups per core on 8 cores.
Projection weights are replicated.
"""

import math
import os
import sys

import numpy as np

sys.path.insert(0, "/opt/trn_rl_repo")

import ml_dtypes

BF16 = ml_dtypes.bfloat16

N = 262144
C_Q = 128
C_KV = 128
H = 4
C = 32
HC = H * C  # 128
NUM_TOKENS = 16384
NCORES = 8
GROUPS = 128          # token groups overall
TPG = 128             # tokens per group
GPC = GROUPS // NCORES  # groups per core = 16
INV_SQRT_C = 1.0 / math.sqrt(C)
PAD_SLOT = 255        # slot id for padding atoms (matches no token slot)

_BUILD_CACHE = {}
LAST_RESULTS = None  # stash of the last BassKernelResults for test harness


def _host_shard(atom_to_token_idx):
    """Assign tokens to 128 LPT-balanced groups of 128 tokens, sort atoms by
    (group, token), and compute the padded layout.

    Returns dict with permutation, destination indices, per-atom slot ids,
    token grid, and cap_tiles."""
    idx = np.asarray(atom_to_token_idx).astype(np.int64)
    counts = np.bincount(idx, minlength=NUM_TOKENS)

    # snake-deal tokens (sorted by size desc) into GROUPS groups
    order_tok = np.argsort(-counts, kind="stable")
    rounds = order_tok.reshape(NUM_TOKENS // GROUPS, GROUPS).copy()
    rounds[1::2] = rounds[1::2, ::-1]
    grp_of_tok = np.empty(NUM_TOKENS, np.int64)
    slot_of_tok = np.empty(NUM_TOKENS, np.int64)
    grp_of_tok[rounds] = np.broadcast_to(
        np.arange(GROUPS)[None, :], rounds.shape
    )
    slot_of_tok[rounds] = np.broadcast_to(
        np.arange(rounds.shape[0])[:, None], rounds.shape
    )
    # token id at (group, slot)
    tok_grid = np.empty((GROUPS, TPG), np.int64)
    tok_grid[grp_of_tok, slot_of_tok] = np.arange(NUM_TOKENS)

    loads = counts[tok_grid].sum(axis=1)  # atoms per group
    cap_tiles = max(1, int(math.ceil(loads.max() / 128.0)))
    cap_atoms = cap_tiles * 128

    # atoms sorted by (group, token id)
    key = grp_of_tok[idx] * NUM_TOKENS + idx
    perm = np.argsort(key, kind="stable")
    gidx = grp_of_tok[idx[perm]]           # nondecreasing group per atom
    group_start = np.searchsorted(gidx, np.arange(GROUPS))
    rank = np.arange(N) - group_start[gidx]
    dest = gidx * cap_atoms + rank         # position in padded atom array
    slots = slot_of_tok[idx[perm]]         # token slot of each (permuted) atom

    return dict(
        perm=perm,
        dest=dest,
        slots=slots,
        tok_grid=tok_grid,
        counts=counts,
        cap_tiles=cap_tiles,
        cap_atoms=cap_atoms,
    )


def _build_nc(cap_tiles, debug_dumps=False, has_bq=False):
    """Build + schedule the SPMD Bass program for a given per-group tile
    capacity. Cached per cap_tiles."""
    key = (cap_tiles, debug_dumps, has_bq)
    if key in _BUILD_CACHE:
        return _BUILD_CACHE[key]

    import concourse.bass as bass
    import concourse.tile as tile
    from concourse import bacc, mybir

    dt = mybir.dt
    AOT = mybir.AluOpType
    AFT = mybir.ActivationFunctionType

    cap_atoms = cap_tiles * 128
    atoms_pc = GPC * cap_atoms         # padded atoms per core
    tiles_pc = GPC * cap_tiles

    nc = bacc.Bacc(
        "TRN2", target_bir_lowering=False, debug=False, num_devices=NCORES
    )

    qxT_d = nc.dram_tensor("qxT", [128, atoms_pc], dt.bfloat16, kind="ExternalInput")
    kvT_d = nc.dram_tensor("kvT", [128, atoms_pc], dt.bfloat16, kind="ExternalInput")
    L_d = nc.dram_tensor("L", [128, tiles_pc], dt.float32, kind="ExternalInput")
    qxoT_d = nc.dram_tensor("qxoT", [128, GPC * TPG], dt.bfloat16, kind="ExternalInput")
    wq_d = nc.dram_tensor("wq", [128, HC], dt.bfloat16, kind="ExternalInput")
    wk_d = nc.dram_tensor("wk", [128, HC], dt.bfloat16, kind="ExternalInput")
    wv_d = nc.dram_tensor("wv", [128, HC], dt.bfloat16, kind="ExternalInput")
    wg_d = nc.dram_tensor("wg", [128, HC], dt.bfloat16, kind="ExternalInput")
    wo_d = nc.dram_tensor("wo", [HC, C_Q], dt.bfloat16, kind="ExternalInput")
    ind_d = nc.dram_tensor("ind", [HC, H], dt.bfloat16, kind="ExternalInput")
    ident_d = nc.dram_tensor("ident", [128, 128], dt.bfloat16, kind="ExternalInput")
    iota_d = nc.dram_tensor("iota", [128, cap_atoms], dt.float32, kind="ExternalInput")
    bq_d = nc.dram_tensor("bqv", [128, 1], dt.float32, kind="ExternalInput")
    out_d = nc.dram_tensor("out", [GPC * TPG, C_Q], dt.float32, kind="ExternalOutput")
    dbg = {}
    if debug_dumps:
        for nm, shp, dty in [
            ("dbg_q2", [128, 512], dt.bfloat16),
            ("dbg_qk", [128, 512], dt.bfloat16),
            ("dbg_a16", [128, 512], dt.bfloat16),
            ("dbg_grp", [128, 132], dt.float32),
            ("dbg_y0", [128, 128], dt.bfloat16),
            ("dbg_gate", [128, 128], dt.bfloat16),
            ("dbg_yT", [128, 128], dt.bfloat16),
        ]:
            dbg[nm] = nc.dram_tensor(nm, shp, dty, kind="ExternalOutput")

    # batches of up to 4 tiles within each group
    batches = []
    b0 = 0
    while b0 < cap_tiles:
        B = min(4, cap_tiles - b0)
        batches.append((b0, B))
        b0 += B

    with tile.TileContext(nc) as tc:
        with (
            tc.tile_pool(name="const", bufs=1) as cpool,
            tc.tile_pool(name="inp", bufs=3) as inp,
            tc.tile_pool(name="sb", bufs=3) as sb,
            tc.tile_pool(name="outp", bufs=2) as outp,
            tc.tile_pool(name="pgrp", bufs=2, space=bass.MemorySpace.PSUM) as pgrp,
            tc.tile_pool(name="pq", bufs=2, space=bass.MemorySpace.PSUM) as pq,
            tc.tile_pool(name="pk", bufs=2, space=bass.MemorySpace.PSUM) as pk,
            tc.tile_pool(name="pv", bufs=2, space=bass.MemorySpace.PSUM) as pv,
        ):
            wq_sb = cpool.tile_from(wq_d[:])
            wk_sb = cpool.tile_from(wk_d[:])
            wv_sb = cpool.tile_from(wv_d[:])
            wg_sb = cpool.tile_from(wg_d[:])
            wo_sb = cpool.tile_from(wo_d[:])
            ind_sb = cpool.tile_from(ind_d[:])
            ident_sb = cpool.tile_from(ident_d[:])
            iota_sb = cpool.tile_from(iota_d[:])
            bq_sb = cpool.tile_from(bq_d[:])
            L_sb = cpool.tile_from(L_d[:])
            qxoT_sb = cpool.tile_from(qxoT_d[:])

            # gate pre-pass: project + sigmoid all 16 groups' tokens up front
            # (keeps ACT's LUT on Exp for the whole main loop, and the dense
            # matmul burst warms the PE clock gate)
            gate_all = cpool.tile([128, GPC * TPG], dt.bfloat16)
            for g in range(GPC):
                g_ps = pq.tile([128, HC], dt.float32, tag="qp")
                nc.tensor.matmul(
                    g_ps[:], qxoT_sb[:, g * TPG : (g + 1) * TPG], wg_sb[:],
                    start=True, stop=True,
                )
                nc.scalar.activation(
                    gate_all[:, g * TPG : (g + 1) * TPG], g_ps[:], AFT.Sigmoid
                )

            for g in range(GPC):
                qx_g = inp.tile([128, cap_atoms], dt.bfloat16, tag="qx")
                nc.sync.dma_start(
                    qx_g[:], qxT_d[:, g * cap_atoms : (g + 1) * cap_atoms]
                )
                kv_g = inp.tile([128, cap_atoms], dt.bfloat16, tag="kv")
                nc.sync.dma_start(
                    kv_g[:], kvT_d[:, g * cap_atoms : (g + 1) * cap_atoms]
                )

                grp_ps = pgrp.tile([128, 132], dt.float32, tag="grp")

                # one-hot [atom, token-slot] for the whole group, one DVE op
                a16 = sb.tile([128, cap_atoms], dt.bfloat16, tag="a16")
                nc.vector.tensor_tensor(
                    a16[:].rearrange("p (t s) -> p t s", t=cap_tiles, s=128),
                    iota_sb[:].rearrange("p (t s) -> p t s", t=cap_tiles, s=128),
                    L_sb[:, g * cap_tiles : (g + 1) * cap_tiles]
                    .unsqueeze(-1)
                    .broadcast_to((128, cap_tiles, 128)),
                    AOT.is_equal,
                )

                for (b0, B) in batches:
                    A = B * 128
                    off = b0 * 128
                    # feature-major q, k for the score chain
                    q_ps = pq.tile([128, A], dt.float32, tag="qp")
                    nc.tensor.matmul(
                        q_ps[:], wq_sb[:], qx_g[:, off : off + A],
                        start=True, stop=True,
                    )
                    k_ps = pk.tile([128, A], dt.float32, tag="kp")
                    nc.tensor.matmul(
                        k_ps[:], wk_sb[:], kv_g[:, off : off + A],
                        start=True, stop=True,
                    )
                    # q2 = q + bq (per-partition bias) on ACT, PSUM -> SBUF;
                    # then qk = q2 * k on DVE (only one PSUM operand allowed)
                    q2 = sb.tile([128, A], dt.bfloat16, tag="q2")
                    if has_bq:
                        nc.scalar.activation(
                            q2[:], q_ps[:], AFT.Identity, bias=bq_sb[:]
                        )
                    else:
                        nc.scalar.activation(q2[:], q_ps[:], AFT.Copy)
                    qk = sb.tile([128, A], dt.bfloat16, tag="qk")
                    nc.vector.tensor_tensor(qk[:], q2[:], k_ps[:], AOT.mult)
                    # s[atom, h] per tile via PE reduction over hc partitions
                    # (shares PSUM slots with q_ps, which is dead by now)
                    s_ps = pq.tile([128, 4 * B], dt.float32, tag="qp")
                    for b in range(B):
                        nc.tensor.matmul(
                            s_ps[:, 4 * b : 4 * b + 4],
                            qk[:, 128 * b : 128 * (b + 1)],
                            ind_sb[:],
                            start=True, stop=True,
                        )
                    # atom-major v
                    v_ps = pv.tile([128, A], dt.float32, tag="vp")
                    for b in range(B):
                        nc.tensor.matmul(
                            v_ps[:, 128 * b : 128 * (b + 1)],
                            kv_g[:, off + 128 * b : off + 128 * (b + 1)],
                            wv_sb[:],
                            start=True, stop=True,
                        )
                    # fused rhs tile: per tile 132 cols = [w (128) | e (4)]
                    # e = exp(s) written straight into the e columns by ACT
                    we = sb.tile([128, B * 132], dt.bfloat16, tag="we")
                    we3 = we[:].rearrange("p (b f) -> p b f", b=B, f=132)
                    nc.scalar.activation(
                        we3[:, :, 128:132],
                        s_ps[:].rearrange("p (b h) -> p b h", b=B, h=H),
                        AFT.Exp,
                    )
                    w_view = we3[:, :, 0:128].rearrange(
                        "p b (h c) -> p b h c", h=H, c=C
                    )
                    v_view = v_ps[:].rearrange("p (b h c) -> p b h c", b=B, h=H, c=C)
                    e_view = (
                        we3[:, :, 128:132]
                        .unsqueeze(-1)
                        .broadcast_to((128, B, H, C))
                    )
                    nc.vector.tensor_tensor(w_view, v_view, e_view, AOT.mult)
                    if debug_dumps and g == 0 and b0 == 0:
                        nc.sync.dma_start(dbg["dbg_q2"][:], q2[:])
                        nc.sync.dma_start(dbg["dbg_qk"][:], qk[:])
                        nc.sync.dma_start(dbg["dbg_a16"][:], a16[:, 0:512])
                    # one fused segment-sum matmul per tile: [num | den]
                    for b in range(B):
                        t = b0 + b
                        nc.tensor.matmul(
                            grp_ps[:],
                            a16[:, 128 * t : 128 * (t + 1)],
                            we[:, 132 * b : 132 * (b + 1)],
                            start=(t == 0), stop=(t == cap_tiles - 1),
                        )

                # ---- group stage: normalize, gate, project out ----
                if debug_dumps and g == 0:
                    grp_cp = sb.tile([128, 132], dt.float32, tag="grpcp")
                    nc.vector.tensor_copy(grp_cp[:], grp_ps[:])
                    nc.sync.dma_start(dbg["dbg_grp"][:], grp_cp[:])
                r32 = sb.tile([128, H], dt.float32, tag="r32")
                nc.vector.reciprocal(r32[:], grp_ps[:, 128:132])
                y0 = sb.tile([128, HC], dt.bfloat16, tag="y0")
                num_view = grp_ps[:, 0:128].rearrange("p (h c) -> p h c", h=H, c=C)
                y0_view = y0[:].rearrange("p (h c) -> p h c", h=H, c=C)
                r_view = r32[:].unsqueeze(-1).broadcast_to((128, H, C))
                nc.vector.tensor_tensor(y0_view, num_view, r_view, AOT.mult)

                y = sb.tile([128, HC], dt.bfloat16, tag="y")
                nc.vector.tensor_tensor(
                    y[:], y0[:], gate_all[:, g * TPG : (g + 1) * TPG], AOT.mult
                )
                yT_ps = pk.tile([128, 128], dt.bfloat16, tag="kp")
                nc.tensor.transpose(yT_ps[:], y[:], ident_sb[:])
                yT16 = sb.tile([128, 128], dt.bfloat16, tag="yT")
                nc.scalar.activation(yT16[:], yT_ps[:], AFT.Copy)
                f_ps = pv.tile([128, C_Q], dt.float32, tag="vp")
                nc.tensor.matmul(
                    f_ps[:], yT16[:], wo_sb[:], start=True, stop=True
                )
                if debug_dumps and g == 0:
                    nc.sync.dma_start(dbg["dbg_y0"][:], y0[:])
                    nc.sync.dma_start(dbg["dbg_gate"][:], gate_all[:, 0:TPG])
                    nc.sync.dma_start(dbg["dbg_yT"][:], yT16[:])
                o32 = outp.tile([128, C_Q], dt.float32, tag="o")
                nc.scalar.activation(o32[:], f_ps[:], AFT.Copy)
                nc.sync.dma_start(out_d[g * TPG : (g + 1) * TPG, :], o32[:])

    nc.compile()
    _BUILD_CACHE[key] = nc
    return nc


def _install_ntff_shim():
    """The agent image's `antenv` lacks `axon_hooks`; recreate it and install
    the ctypes NTFF profile hook the way trn_agent_boot would."""
    import types

    import antenv

    if "antenv.axon_hooks" in sys.modules:
        return
    mod = types.ModuleType("antenv.axon_hooks")
    holder = [None]
    mod.set_axon_ntff_profile_hook = lambda h: holder.__setitem__(0, h)
    mod.get_axon_ntff_profile_hook = lambda: holder[0]
    sys.modules["antenv.axon_hooks"] = mod
    antenv.axon_hooks = mod
    try:
        sys.path.insert(0, "/root/.axon_site")
        from trn_agent_boot.trn_boot import _ntff_profile_via_ctypes

        hook = _ntff_profile_via_ctypes("/opt/axon/libaxon_pjrt.so")
        mod.set_axon_ntff_profile_hook(hook)
    except Exception as e:  # degrade to no tracing
        print(f"ntff shim install failed: {e}")


def kernel(q_x, kv_x, atom_to_token_idx, Wq, bq, Wk, Wv, Wg, Wo, bo):
    global LAST_RESULTS
    from concourse.bass_utils import run_bass_kernel_spmd

    q_x = np.asarray(q_x, np.float32)
    kv_x = np.asarray(kv_x, np.float32)
    Wq = np.asarray(Wq, np.float32)
    bq = np.asarray(bq, np.float32)
    Wk = np.asarray(Wk, np.float32)
    Wv = np.asarray(Wv, np.float32)
    Wg = np.asarray(Wg, np.float32)
    Wo = np.asarray(Wo, np.float32)
    bo = np.asarray(bo, np.float32)

    sh = _host_shard(atom_to_token_idx)
    cap_tiles = sh["cap_tiles"]
    cap_atoms = sh["cap_atoms"]
    perm, dest, slots = sh["perm"], sh["dest"], sh["slots"]
    tok_grid = sh["tok_grid"]

    # padded, permuted, bf16 inputs
    tot = GROUPS * cap_atoms
    Xq = np.zeros((tot, 128), BF16)
    Xq[dest] = q_x[perm].astype(BF16)
    Xkv = np.zeros((tot, 128), BF16)
    Xkv[dest] = kv_x[perm].astype(BF16)
    Lfull = np.full(tot, PAD_SLOT, np.float32)
    Lfull[dest] = slots.astype(np.float32)

    wq_h = (Wq * INV_SQRT_C).astype(BF16)
    wk_h = (Wk * INV_SQRT_C).astype(BF16)
    wv_h = Wv.astype(BF16)
    wg_h = Wg.astype(BF16)
    wo_h = Wo.astype(BF16)
    bq_h = (bq * INV_SQRT_C).astype(np.float32).reshape(128, 1)
    ind_h = np.zeros((HC, H), BF16)
    for h in range(H):
        ind_h[h * C : (h + 1) * C, h] = 1
    ident_h = np.eye(128, dtype=BF16)
    iota_h = np.broadcast_to(
        np.tile(np.arange(128, dtype=np.float32), cap_tiles), (128, cap_atoms)
    ).copy()

    apc = GPC * cap_atoms
    in_maps = []
    for c in range(NCORES):
        rows = slice(c * apc, (c + 1) * apc)
        qxT = np.ascontiguousarray(Xq[rows].T)
        kvT = np.ascontiguousarray(Xkv[rows].T)
        Lc = np.ascontiguousarray(
            Lfull[rows].reshape(GPC * cap_tiles, 128).T
        )
        tok_core = tok_grid[c * GPC : (c + 1) * GPC].reshape(GPC * TPG)
        qxoT = np.ascontiguousarray(q_x[tok_core].T.astype(BF16))
        in_maps.append(
            dict(
                qxT=qxT, kvT=kvT, L=Lc, qxoT=qxoT,
                wq=wq_h, wk=wk_h, wv=wv_h, wg=wg_h, wo=wo_h,
                ind=ind_h, ident=ident_h, iota=iota_h, bqv=bq_h,
            )
        )

    nc = _build_nc(
        cap_tiles,
        debug_dumps=os.environ.get("KERNEL_DEBUG_DUMPS", "0") == "1",
        has_bq=bool(np.any(bq != 0)),
    )
    trace = os.environ.get("KERNEL_TRACE", "0") == "1"
    if trace:
        _install_ntff_shim()
    res = run_bass_kernel_spmd(
        nc, in_maps, list(range(NCORES)), trace=trace,
        tmpdir=os.environ.get("KERNEL_TRACE_DIR") or None,
    )
    LAST_RESULTS = res

    out_full = np.broadcast_to(bo, (N, C_Q)).astype(np.float32).copy()
    for c in range(NCORES):
        tok_core = tok_grid[c * GPC : (c + 1) * GPC].reshape(GPC * TPG)
        out_full[tok_core] = res.results[c]["out"] + bo
    empty = np.where(sh["counts"] == 0)[0]
    if empty.size:
        out_full[empty] = bo
    return out_full


# revision 48
# speedup vs baseline: 2.7007x; 1.1317x over previous
"""
Trainium2 Bass kernel for nn_Local_Attention (segment-softmax attention over
atoms grouped into tokens).

Algorithm notes (reference semantics):
  q = (q_x @ Wq + bq) / sqrt(C)            [N, H*C]
  k = kv_x @ Wk ; v = kv_x @ Wv            [N, H*C]
  s[i,h] = sum_c q[i,h,c] k[i,h,c] / sqrt(C)
  alpha  = softmax of s over atoms sharing a token (segment softmax)
  out[t] = sum_{i in t} alpha[i] * v[i]    (only rows t < NUM_TOKENS nonzero)
  result = (out * sigmoid(q_x @ Wg)) @ Wo + bo

Key simplifications used here:
  * Scores are tiny (|s| < ~0.1), so the segment-max subtraction is skipped:
    alpha = e / segsum(e), e = exp(s). Numerator and denominator are both
    segment *sums*, and the division happens at token level:
    out[t] = segsum(e*v)[t] / segsum(e)[t].
  * Rows >= NUM_TOKENS of the result equal bo (segment sum there is zero), so
    only the first NUM_TOKENS rows are computed on device.
  * Segment sums are computed as one-hot matmuls: the host sorts atoms by
    token, packs 128 tokens per "group" (LPT-balanced), pads each group's
    atom list to a fixed tile capacity, and each 128-atom tile contributes
    via a [atom, token-slot] one-hot built on device from per-atom slot ids.

Sharding: 128 groups of 128 tokens each; 16 groups per core on 8 cores.
Projection weights are replicated.
"""

import math
import os
import sys

import numpy as np

sys.path.insert(0, "/opt/trn_rl_repo")

import ml_dtypes

BF16 = ml_dtypes.bfloat16

N = 262144
C_Q = 128
C_KV = 128
H = 4
C = 32
HC = H * C  # 128
NUM_TOKENS = 16384
NCORES = 8
GROUPS = 128          # token groups overall
TPG = 128             # tokens per group
GPC = GROUPS // NCORES  # groups per core = 16
INV_SQRT_C = 1.0 / math.sqrt(C)
PAD_SLOT = 255        # slot id for padding atoms (matches no token slot)

_BUILD_CACHE = {}
LAST_RESULTS = None  # stash of the last BassKernelResults for test harness


def _host_shard(atom_to_token_idx):
    """Assign tokens to 128 LPT-balanced groups of 128 tokens, sort atoms by
    (group, token), and compute the padded layout.

    Returns dict with permutation, destination indices, per-atom slot ids,
    token grid, and cap_tiles."""
    idx = np.asarray(atom_to_token_idx).astype(np.int64)
    counts = np.bincount(idx, minlength=NUM_TOKENS)

    # snake-deal tokens (sorted by size desc) into GROUPS groups
    order_tok = np.argsort(-counts, kind="stable")
    rounds = order_tok.reshape(NUM_TOKENS // GROUPS, GROUPS).copy()
    rounds[1::2] = rounds[1::2, ::-1]
    grp_of_tok = np.empty(NUM_TOKENS, np.int64)
    slot_of_tok = np.empty(NUM_TOKENS, np.int64)
    grp_of_tok[rounds] = np.broadcast_to(
        np.arange(GROUPS)[None, :], rounds.shape
    )
    slot_of_tok[rounds] = np.broadcast_to(
        np.arange(rounds.shape[0])[:, None], rounds.shape
    )
    # token id at (group, slot)
    tok_grid = np.empty((GROUPS, TPG), np.int64)
    tok_grid[grp_of_tok, slot_of_tok] = np.arange(NUM_TOKENS)

    loads = counts[tok_grid].sum(axis=1)  # atoms per group
    cap_tiles = max(1, int(math.ceil(loads.max() / 128.0)))
    cap_atoms = cap_tiles * 128

    # atoms sorted by (group, token id)
    key = grp_of_tok[idx] * NUM_TOKENS + idx
    perm = np.argsort(key, kind="stable")
    gidx = grp_of_tok[idx[perm]]           # nondecreasing group per atom
    group_start = np.searchsorted(gidx, np.arange(GROUPS))
    rank = np.arange(N) - group_start[gidx]
    dest = gidx * cap_atoms + rank         # position in padded atom array
    slots = slot_of_tok[idx[perm]]         # token slot of each (permuted) atom

    return dict(
        perm=perm,
        dest=dest,
        slots=slots,
        tok_grid=tok_grid,
        counts=counts,
        cap_tiles=cap_tiles,
        cap_atoms=cap_atoms,
    )


def _build_nc(cap_tiles, debug_dumps=False, has_bq=False):
    """Build + schedule the SPMD Bass program for a given per-group tile
    capacity. Cached per cap_tiles."""
    key = (cap_tiles, debug_dumps, has_bq)
    if key in _BUILD_CACHE:
        return _BUILD_CACHE[key]

    import concourse.bass as bass
    import concourse.tile as tile
    from concourse import bacc, mybir

    dt = mybir.dt
    AOT = mybir.AluOpType
    AFT = mybir.ActivationFunctionType

    cap_atoms = cap_tiles * 128
    atoms_pc = GPC * cap_atoms         # padded atoms per core
    tiles_pc = GPC * cap_tiles

    nc = bacc.Bacc(
        "TRN2", target_bir_lowering=False, debug=False, num_devices=NCORES
    )

    qxT_d = nc.dram_tensor("qxT", [128, atoms_pc], dt.bfloat16, kind="ExternalInput")
    kvT_d = nc.dram_tensor("kvT", [128, atoms_pc], dt.bfloat16, kind="ExternalInput")
    qxoT_d = nc.dram_tensor("qxoT", [128, GPC * TPG], dt.bfloat16, kind="ExternalInput")
    wq_d = nc.dram_tensor("wq", [128, HC], dt.bfloat16, kind="ExternalInput")
    wk_d = nc.dram_tensor("wk", [128, HC], dt.bfloat16, kind="ExternalInput")
    wv_d = nc.dram_tensor("wv", [128, HC], dt.bfloat16, kind="ExternalInput")
    wg_d = nc.dram_tensor("wg", [128, HC], dt.bfloat16, kind="ExternalInput")
    wo_d = nc.dram_tensor("wo", [HC, C_Q], dt.bfloat16, kind="ExternalInput")
    ind_d = nc.dram_tensor("ind", [HC, H], dt.bfloat16, kind="ExternalInput")
    ident_d = nc.dram_tensor("ident", [128, 128], dt.bfloat16, kind="ExternalInput")
    a_d = nc.dram_tensor("aT", [128, atoms_pc], dt.bfloat16, kind="ExternalInput")
    bq_d = nc.dram_tensor("bqv", [128, 1], dt.float32, kind="ExternalInput")
    out_d = nc.dram_tensor("out", [GPC * TPG, C_Q], dt.float32, kind="ExternalOutput")
    dbg = {}
    if debug_dumps:
        for nm, shp, dty in [
            ("dbg_q2", [128, 512], dt.bfloat16),
            ("dbg_qk", [128, 512], dt.bfloat16),
            ("dbg_a16", [128, 512], dt.bfloat16),
            ("dbg_grp", [128, 132], dt.float32),
            ("dbg_y0", [128, 128], dt.bfloat16),
            ("dbg_gate", [128, 128], dt.bfloat16),
            ("dbg_yT", [128, 128], dt.bfloat16),
        ]:
            dbg[nm] = nc.dram_tensor(nm, shp, dty, kind="ExternalOutput")

    # batches of up to 4 tiles within each group
    batches = []
    b0 = 0
    while b0 < cap_tiles:
        B = min(4, cap_tiles - b0)
        batches.append((b0, B))
        b0 += B

    with tile.TileContext(nc) as tc:
        with (
            tc.tile_pool(name="const", bufs=1) as cpool,
            tc.tile_pool(name="inp", bufs=3) as inp,
            tc.tile_pool(name="sb", bufs=3) as sb,
            tc.tile_pool(name="outp", bufs=2) as outp,
            tc.tile_pool(name="pgrp", bufs=2, space=bass.MemorySpace.PSUM) as pgrp,
            tc.tile_pool(name="pq", bufs=2, space=bass.MemorySpace.PSUM) as pq,
            tc.tile_pool(name="pk", bufs=2, space=bass.MemorySpace.PSUM) as pk,
            tc.tile_pool(name="pv", bufs=2, space=bass.MemorySpace.PSUM) as pv,
        ):
            wq_sb = cpool.tile_from(wq_d[:])
            wk_sb = cpool.tile_from(wk_d[:])
            wv_sb = cpool.tile_from(wv_d[:])
            wg_sb = cpool.tile_from(wg_d[:])
            wo_sb = cpool.tile_from(wo_d[:])
            ind_sb = cpool.tile_from(ind_d[:])
            ident_sb = cpool.tile_from(ident_d[:])
            bq_sb = cpool.tile_from(bq_d[:])
            qxoT_sb = cpool.tile_from(qxoT_d[:])

            # gate pre-pass: project + sigmoid all 16 groups' tokens up front
            # (keeps ACT's LUT on Exp for the whole main loop, and the dense
            # matmul burst warms the PE clock gate)
            gate_all = cpool.tile([128, GPC * TPG], dt.bfloat16)
            for g in range(GPC):
                g_ps = pq.tile([128, HC], dt.float32, tag="qp")
                nc.tensor.matmul(
                    g_ps[:], qxoT_sb[:, g * TPG : (g + 1) * TPG], wg_sb[:],
                    start=True, stop=True,
                )
                nc.scalar.activation(
                    gate_all[:, g * TPG : (g + 1) * TPG], g_ps[:], AFT.Sigmoid
                )

            for g in range(GPC):
                qx_g = inp.tile([128, cap_atoms], dt.bfloat16, tag="qx")
                nc.sync.dma_start(
                    qx_g[:], qxT_d[:, g * cap_atoms : (g + 1) * cap_atoms]
                )
                kv_g = inp.tile([128, cap_atoms], dt.bfloat16, tag="kv")
                nc.sync.dma_start(
                    kv_g[:], kvT_d[:, g * cap_atoms : (g + 1) * cap_atoms]
                )

                grp_ps = pgrp.tile([128, 132], dt.float32, tag="grp")

                # one-hot [atom, token-slot], host-precomputed, via DMA
                a16 = inp.tile([128, cap_atoms], dt.bfloat16, tag="a")
                nc.sync.dma_start(
                    a16[:], a_d[:, g * cap_atoms : (g + 1) * cap_atoms]
                )

                for (b0, B) in batches:
                    A = B * 128
                    off = b0 * 128
                    # feature-major q, k for the score chain
                    q_ps = pq.tile([128, A], dt.float32, tag="qp")
                    nc.tensor.matmul(
                        q_ps[:], wq_sb[:], qx_g[:, off : off + A],
                        start=True, stop=True,
                    )
                    k_ps = pk.tile([128, A], dt.float32, tag="kp")
                    nc.tensor.matmul(
                        k_ps[:], wk_sb[:], kv_g[:, off : off + A],
                        start=True, stop=True,
                    )
                    # q2 = q + bq (per-partition bias) on ACT, PSUM -> SBUF;
                    # then qk = q2 * k on DVE (only one PSUM operand allowed)
                    q2 = sb.tile([128, A], dt.bfloat16, tag="q2")
                    if has_bq:
                        nc.scalar.activation(
                            q2[:], q_ps[:], AFT.Identity, bias=bq_sb[:]
                        )
                    else:
                        nc.scalar.activation(q2[:], q_ps[:], AFT.Copy)
                    qk = sb.tile([128, A], dt.bfloat16, tag="qk")
                    nc.vector.tensor_tensor(qk[:], q2[:], k_ps[:], AOT.mult)
                    # s[atom, h] per tile via PE reduction over hc partitions
                    # (shares PSUM slots with q_ps, which is dead by now)
                    s_ps = pq.tile([128, 4 * B], dt.float32, tag="qp")
                    for b in range(B):
                        nc.tensor.matmul(
                            s_ps[:, 4 * b : 4 * b + 4],
                            qk[:, 128 * b : 128 * (b + 1)],
                            ind_sb[:],
                            start=True, stop=True,
                        )
                    # atom-major v
                    v_ps = pv.tile([128, A], dt.float32, tag="vp")
                    for b in range(B):
                        nc.tensor.matmul(
                            v_ps[:, 128 * b : 128 * (b + 1)],
                            kv_g[:, off + 128 * b : off + 128 * (b + 1)],
                            wv_sb[:],
                            start=True, stop=True,
                        )
                    # fused rhs tile: per tile 132 cols = [w (128) | e (4)]
                    # e = exp(s) written straight into the e columns by ACT
                    we = sb.tile([128, B * 132], dt.bfloat16, tag="we")
                    we3 = we[:].rearrange("p (b f) -> p b f", b=B, f=132)
                    nc.scalar.activation(
                        we3[:, :, 128:132],
                        s_ps[:].rearrange("p (b h) -> p b h", b=B, h=H),
                        AFT.Exp,
                    )
                    w_view = we3[:, :, 0:128].rearrange(
                        "p b (h c) -> p b h c", h=H, c=C
                    )
                    v_view = v_ps[:].rearrange("p (b h c) -> p b h c", b=B, h=H, c=C)
                    e_view = (
                        we3[:, :, 128:132]
                        .unsqueeze(-1)
                        .broadcast_to((128, B, H, C))
                    )
                    nc.vector.tensor_tensor(w_view, v_view, e_view, AOT.mult)
                    if debug_dumps and g == 0 and b0 == 0:
                        nc.sync.dma_start(dbg["dbg_q2"][:], q2[:])
                        nc.sync.dma_start(dbg["dbg_qk"][:], qk[:])
                        nc.sync.dma_start(dbg["dbg_a16"][:], a16[:, 0:512])
                    # one fused segment-sum matmul per tile: [num | den]
                    for b in range(B):
                        t = b0 + b
                        nc.tensor.matmul(
                            grp_ps[:],
                            a16[:, 128 * t : 128 * (t + 1)],
                            we[:, 132 * b : 132 * (b + 1)],
                            start=(t == 0), stop=(t == cap_tiles - 1),
                        )

                # ---- group stage: normalize, gate, project out ----
                if debug_dumps and g == 0:
                    grp_cp = sb.tile([128, 132], dt.float32, tag="grpcp")
                    nc.vector.tensor_copy(grp_cp[:], grp_ps[:])
                    nc.sync.dma_start(dbg["dbg_grp"][:], grp_cp[:])
                r32 = sb.tile([128, H], dt.float32, tag="r32")
                nc.vector.reciprocal(r32[:], grp_ps[:, 128:132])
                y0 = sb.tile([128, HC], dt.bfloat16, tag="y0")
                num_view = grp_ps[:, 0:128].rearrange("p (h c) -> p h c", h=H, c=C)
                y0_view = y0[:].rearrange("p (h c) -> p h c", h=H, c=C)
                r_view = r32[:].unsqueeze(-1).broadcast_to((128, H, C))
                nc.vector.tensor_tensor(y0_view, num_view, r_view, AOT.mult)

                y = sb.tile([128, HC], dt.bfloat16, tag="y")
                nc.vector.tensor_tensor(
                    y[:], y0[:], gate_all[:, g * TPG : (g + 1) * TPG], AOT.mult
                )
                yT_ps = pk.tile([128, 128], dt.bfloat16, tag="kp")
                nc.tensor.transpose(yT_ps[:], y[:], ident_sb[:])
                yT16 = sb.tile([128, 128], dt.bfloat16, tag="yT")
                nc.scalar.activation(yT16[:], yT_ps[:], AFT.Copy)
                f_ps = pv.tile([128, C_Q], dt.float32, tag="vp")
                nc.tensor.matmul(
                    f_ps[:], yT16[:], wo_sb[:], start=True, stop=True
                )
                if debug_dumps and g == 0:
                    nc.sync.dma_start(dbg["dbg_y0"][:], y0[:])
                    nc.sync.dma_start(dbg["dbg_gate"][:], gate_all[:, 0:TPG])
                    nc.sync.dma_start(dbg["dbg_yT"][:], yT16[:])
                o32 = outp.tile([128, C_Q], dt.float32, tag="o")
                nc.scalar.activation(o32[:], f_ps[:], AFT.Copy)
                nc.sync.dma_start(out_d[g * TPG : (g + 1) * TPG, :], o32[:])

    nc.compile()
    _BUILD_CACHE[key] = nc
    return nc


def _install_ntff_shim():
    """The agent image's `antenv` lacks `axon_hooks`; recreate it and install
    the ctypes NTFF profile hook the way trn_agent_boot would."""
    import types

    import antenv

    if "antenv.axon_hooks" in sys.modules:
        return
    mod = types.ModuleType("antenv.axon_hooks")
    holder = [None]
    mod.set_axon_ntff_profile_hook = lambda h: holder.__setitem__(0, h)
    mod.get_axon_ntff_profile_hook = lambda: holder[0]
    sys.modules["antenv.axon_hooks"] = mod
    antenv.axon_hooks = mod
    try:
        sys.path.insert(0, "/root/.axon_site")
        from trn_agent_boot.trn_boot import _ntff_profile_via_ctypes

        hook = _ntff_profile_via_ctypes("/opt/axon/libaxon_pjrt.so")
        mod.set_axon_ntff_profile_hook(hook)
    except Exception as e:  # degrade to no tracing
        print(f"ntff shim install failed: {e}")


def kernel(q_x, kv_x, atom_to_token_idx, Wq, bq, Wk, Wv, Wg, Wo, bo):
    global LAST_RESULTS
    from concourse.bass_utils import run_bass_kernel_spmd

    q_x = np.asarray(q_x, np.float32)
    kv_x = np.asarray(kv_x, np.float32)
    Wq = np.asarray(Wq, np.float32)
    bq = np.asarray(bq, np.float32)
    Wk = np.asarray(Wk, np.float32)
    Wv = np.asarray(Wv, np.float32)
    Wg = np.asarray(Wg, np.float32)
    Wo = np.asarray(Wo, np.float32)
    bo = np.asarray(bo, np.float32)

    sh = _host_shard(atom_to_token_idx)
    cap_tiles = sh["cap_tiles"]
    cap_atoms = sh["cap_atoms"]
    perm, dest, slots = sh["perm"], sh["dest"], sh["slots"]
    tok_grid = sh["tok_grid"]

    # padded, permuted, bf16 inputs
    tot = GROUPS * cap_atoms
    Xq = np.zeros((tot, 128), BF16)
    Xq[dest] = q_x[perm].astype(BF16)
    Xkv = np.zeros((tot, 128), BF16)
    Xkv[dest] = kv_x[perm].astype(BF16)
    Afull = np.zeros((tot, TPG), BF16)
    Afull[dest, slots] = 1

    wq_h = (Wq * INV_SQRT_C).astype(BF16)
    wk_h = (Wk * INV_SQRT_C).astype(BF16)
    wv_h = Wv.astype(BF16)
    wg_h = Wg.astype(BF16)
    wo_h = Wo.astype(BF16)
    bq_h = (bq * INV_SQRT_C).astype(np.float32).reshape(128, 1)
    ind_h = np.zeros((HC, H), BF16)
    for h in range(H):
        ind_h[h * C : (h + 1) * C, h] = 1
    ident_h = np.eye(128, dtype=BF16)

    apc = GPC * cap_atoms
    in_maps = []
    for c in range(NCORES):
        rows = slice(c * apc, (c + 1) * apc)
        qxT = np.ascontiguousarray(Xq[rows].T)
        kvT = np.ascontiguousarray(Xkv[rows].T)
        aT = np.ascontiguousarray(
            Afull[rows]
            .reshape(GPC * cap_tiles, 128, TPG)
            .transpose(1, 0, 2)
            .reshape(128, apc)
        )
        tok_core = tok_grid[c * GPC : (c + 1) * GPC].reshape(GPC * TPG)
        qxoT = np.ascontiguousarray(q_x[tok_core].T.astype(BF16))
        in_maps.append(
            dict(
                qxT=qxT, kvT=kvT, aT=aT, qxoT=qxoT,
                wq=wq_h, wk=wk_h, wv=wv_h, wg=wg_h, wo=wo_h,
                ind=ind_h, ident=ident_h, bqv=bq_h,
            )
        )

    nc = _build_nc(
        cap_tiles,
        debug_dumps=os.environ.get("KERNEL_DEBUG_DUMPS", "0") == "1",
        has_bq=bool(np.any(bq != 0)),
    )
    trace = os.environ.get("KERNEL_TRACE", "0") == "1"
    if trace:
        _install_ntff_shim()
    res = run_bass_kernel_spmd(
        nc, in_maps, list(range(NCORES)), trace=trace,
        tmpdir=os.environ.get("KERNEL_TRACE_DIR") or None,
    )
    LAST_RESULTS = res

    out_full = np.broadcast_to(bo, (N, C_Q)).astype(np.float32).copy()
    for c in range(NCORES):
        tok_core = tok_grid[c * GPC : (c + 1) * GPC].reshape(GPC * TPG)
        out_full[tok_core] = res.results[c]["out"] + bo
    empty = np.where(sh["counts"] == 0)[0]
    if empty.size:
        out_full[empty] = bo
    return out_full


# revision 50
# speedup vs baseline: 2.7662x; 1.0242x over previous
"""
Trainium2 Bass kernel for nn_Local_Attention (segment-softmax attention over
atoms grouped into tokens).

Algorithm notes (reference semantics):
  q = (q_x @ Wq + bq) / sqrt(C)            [N, H*C]
  k = kv_x @ Wk ; v = kv_x @ Wv            [N, H*C]
  s[i,h] = sum_c q[i,h,c] k[i,h,c] / sqrt(C)
  alpha  = softmax of s over atoms sharing a token (segment softmax)
  out[t] = sum_{i in t} alpha[i] * v[i]    (only rows t < NUM_TOKENS nonzero)
  result = (out * sigmoid(q_x @ Wg)) @ Wo + bo

Key simplifications used here:
  * Scores are tiny (|s| < ~0.1), so the segment-max subtraction is skipped:
    alpha = e / segsum(e), e = exp(s). Numerator and denominator are both
    segment *sums*, and the division happens at token level:
    out[t] = segsum(e*v)[t] / segsum(e)[t].
  * Rows >= NUM_TOKENS of the result equal bo (segment sum there is zero), so
    only the first NUM_TOKENS rows are computed on device.
  * Segment sums are computed as one-hot matmuls: the host sorts atoms by
    token, packs 128 tokens per "group" (LPT-balanced), pads each group's
    atom list to a fixed tile capacity, and each 128-atom tile contributes
    via a [atom, token-slot] one-hot built on device from per-atom slot ids.

Sharding: 128 groups of 128 tokens each; 16 groups per core on 8 cores.
Projection weights are replicated.
"""

import math
import os
import sys

import numpy as np

sys.path.insert(0, "/opt/trn_rl_repo")

import ml_dtypes

BF16 = ml_dtypes.bfloat16

N = 262144
C_Q = 128
C_KV = 128
H = 4
C = 32
HC = H * C  # 128
NUM_TOKENS = 16384
NCORES = 8
GROUPS = 128          # token groups overall
TPG = 128             # tokens per group
GPC = GROUPS // NCORES  # groups per core = 16
INV_SQRT_C = 1.0 / math.sqrt(C)
PAD_SLOT = 255        # slot id for padding atoms (matches no token slot)

_BUILD_CACHE = {}
LAST_RESULTS = None  # stash of the last BassKernelResults for test harness


def _host_shard(atom_to_token_idx):
    """Assign tokens to 128 LPT-balanced groups of 128 tokens, sort atoms by
    (group, token), and compute the padded layout.

    Returns dict with permutation, destination indices, per-atom slot ids,
    token grid, and cap_tiles."""
    idx = np.asarray(atom_to_token_idx).astype(np.int64)
    counts = np.bincount(idx, minlength=NUM_TOKENS)

    # snake-deal tokens (sorted by size desc) into GROUPS groups
    order_tok = np.argsort(-counts, kind="stable")
    rounds = order_tok.reshape(NUM_TOKENS // GROUPS, GROUPS).copy()
    rounds[1::2] = rounds[1::2, ::-1]
    grp_of_tok = np.empty(NUM_TOKENS, np.int64)
    slot_of_tok = np.empty(NUM_TOKENS, np.int64)
    grp_of_tok[rounds] = np.broadcast_to(
        np.arange(GROUPS)[None, :], rounds.shape
    )
    slot_of_tok[rounds] = np.broadcast_to(
        np.arange(rounds.shape[0])[:, None], rounds.shape
    )
    # token id at (group, slot)
    tok_grid = np.empty((GROUPS, TPG), np.int64)
    tok_grid[grp_of_tok, slot_of_tok] = np.arange(NUM_TOKENS)

    loads = counts[tok_grid].sum(axis=1)  # atoms per group
    cap_tiles = max(1, int(math.ceil(loads.max() / 128.0)))
    cap_atoms = cap_tiles * 128

    # atoms sorted by (group, token id)
    key = grp_of_tok[idx] * NUM_TOKENS + idx
    perm = np.argsort(key, kind="stable")
    gidx = grp_of_tok[idx[perm]]           # nondecreasing group per atom
    group_start = np.searchsorted(gidx, np.arange(GROUPS))
    rank = np.arange(N) - group_start[gidx]
    dest = gidx * cap_atoms + rank         # position in padded atom array
    slots = slot_of_tok[idx[perm]]         # token slot of each (permuted) atom

    return dict(
        perm=perm,
        dest=dest,
        slots=slots,
        tok_grid=tok_grid,
        counts=counts,
        cap_tiles=cap_tiles,
        cap_atoms=cap_atoms,
    )


def _build_nc(cap_tiles, debug_dumps=False, has_bq=False):
    """Build + schedule the SPMD Bass program for a given per-group tile
    capacity. Cached per cap_tiles."""
    key = (cap_tiles, debug_dumps, has_bq)
    if key in _BUILD_CACHE:
        return _BUILD_CACHE[key]

    import concourse.bass as bass
    import concourse.tile as tile
    from concourse import bacc, mybir

    dt = mybir.dt
    AOT = mybir.AluOpType
    AFT = mybir.ActivationFunctionType

    cap_atoms = cap_tiles * 128
    atoms_pc = GPC * cap_atoms         # padded atoms per core
    tiles_pc = GPC * cap_tiles

    nc = bacc.Bacc(
        "TRN2", target_bir_lowering=False, debug=False, num_devices=NCORES
    )

    qxT_d = nc.dram_tensor("qxT", [128, atoms_pc], dt.bfloat16, kind="ExternalInput")
    kvT_d = nc.dram_tensor("kvT", [128, atoms_pc], dt.bfloat16, kind="ExternalInput")
    qxoT_d = nc.dram_tensor("qxoT", [128, GPC * TPG], dt.bfloat16, kind="ExternalInput")
    wq_d = nc.dram_tensor("wq", [128, HC], dt.bfloat16, kind="ExternalInput")
    wk_d = nc.dram_tensor("wk", [128, HC], dt.bfloat16, kind="ExternalInput")
    wv_d = nc.dram_tensor("wv", [128, HC], dt.bfloat16, kind="ExternalInput")
    wg_d = nc.dram_tensor("wg", [128, HC], dt.bfloat16, kind="ExternalInput")
    wo_d = nc.dram_tensor("wo", [HC, C_Q], dt.bfloat16, kind="ExternalInput")
    ind_d = nc.dram_tensor("ind", [HC, H], dt.bfloat16, kind="ExternalInput")
    ident_d = nc.dram_tensor("ident", [128, 128], dt.bfloat16, kind="ExternalInput")
    a_d = nc.dram_tensor("aT", [128, atoms_pc], dt.bfloat16, kind="ExternalInput")
    bq_d = nc.dram_tensor("bqv", [128, 1], dt.float32, kind="ExternalInput")
    out_d = nc.dram_tensor("out", [GPC * TPG, C_Q], dt.float32, kind="ExternalOutput")
    dbg = {}
    if debug_dumps:
        for nm, shp, dty in [
            ("dbg_q2", [128, 512], dt.bfloat16),
            ("dbg_qk", [128, 512], dt.bfloat16),
            ("dbg_a16", [128, 512], dt.bfloat16),
            ("dbg_grp", [128, 132], dt.float32),
            ("dbg_y0", [128, 128], dt.bfloat16),
            ("dbg_gate", [128, 128], dt.bfloat16),
            ("dbg_yT", [128, 128], dt.bfloat16),
        ]:
            dbg[nm] = nc.dram_tensor(nm, shp, dty, kind="ExternalOutput")

    # batches of up to 4 tiles within each group
    batches = []
    b0 = 0
    while b0 < cap_tiles:
        B = min(4, cap_tiles - b0)
        batches.append((b0, B))
        b0 += B

    with tile.TileContext(nc) as tc:
        with (
            tc.tile_pool(name="const", bufs=1) as cpool,
            tc.tile_pool(name="inp", bufs=3) as inp,
            tc.tile_pool(name="sb", bufs=3) as sb,
            tc.tile_pool(name="outp", bufs=2) as outp,
            tc.tile_pool(name="pgrp", bufs=2, space=bass.MemorySpace.PSUM) as pgrp,
            tc.tile_pool(name="pq", bufs=2, space=bass.MemorySpace.PSUM) as pq,
            tc.tile_pool(name="pk", bufs=2, space=bass.MemorySpace.PSUM) as pk,
            tc.tile_pool(name="pv", bufs=2, space=bass.MemorySpace.PSUM) as pv,
        ):
            wq_sb = cpool.tile_from(wq_d[:])
            wk_sb = cpool.tile_from(wk_d[:])
            wv_sb = cpool.tile_from(wv_d[:])
            wg_sb = cpool.tile_from(wg_d[:])
            wo_sb = cpool.tile_from(wo_d[:])
            ind_sb = cpool.tile_from(ind_d[:])
            ident_sb = cpool.tile_from(ident_d[:])
            bq_sb = cpool.tile_from(bq_d[:])
            qxoT_sb = cpool.tile_from(qxoT_d[:])

            # gate pre-pass: project + sigmoid all 16 groups' tokens up front
            # (keeps ACT's LUT on Exp for the whole main loop, and the dense
            # matmul burst warms the PE clock gate)
            gate_all = cpool.tile([128, GPC * TPG], dt.bfloat16)
            y_all = cpool.tile([128, GPC * TPG], dt.bfloat16)
            for g in range(GPC):
                g_ps = pq.tile([128, HC], dt.float32, tag="qp")
                nc.tensor.matmul(
                    g_ps[:], qxoT_sb[:, g * TPG : (g + 1) * TPG], wg_sb[:],
                    start=True, stop=True,
                )
                nc.scalar.activation(
                    gate_all[:, g * TPG : (g + 1) * TPG], g_ps[:], AFT.Sigmoid
                )

            for g in range(GPC):
                qx_g = inp.tile([128, cap_atoms], dt.bfloat16, tag="qx")
                nc.sync.dma_start(
                    qx_g[:], qxT_d[:, g * cap_atoms : (g + 1) * cap_atoms]
                )
                kv_g = inp.tile([128, cap_atoms], dt.bfloat16, tag="kv")
                nc.sync.dma_start(
                    kv_g[:], kvT_d[:, g * cap_atoms : (g + 1) * cap_atoms]
                )

                grp_ps = pgrp.tile([128, 132], dt.float32, tag="grp")

                # one-hot [atom, token-slot], host-precomputed, via DMA
                a16 = inp.tile([128, cap_atoms], dt.bfloat16, tag="a")
                nc.sync.dma_start(
                    a16[:], a_d[:, g * cap_atoms : (g + 1) * cap_atoms]
                )

                for (b0, B) in batches:
                    A = B * 128
                    off = b0 * 128
                    # feature-major q, k for the score chain
                    q_ps = pq.tile([128, A], dt.float32, tag="qp")
                    nc.tensor.matmul(
                        q_ps[:], wq_sb[:], qx_g[:, off : off + A],
                        start=True, stop=True,
                    )
                    k_ps = pk.tile([128, A], dt.float32, tag="kp")
                    nc.tensor.matmul(
                        k_ps[:], wk_sb[:], kv_g[:, off : off + A],
                        start=True, stop=True,
                    )
                    # q2 = q + bq (per-partition bias) on ACT, PSUM -> SBUF;
                    # then qk = q2 * k on DVE (only one PSUM operand allowed)
                    q2 = sb.tile([128, A], dt.bfloat16, tag="q2")
                    if has_bq:
                        nc.scalar.activation(
                            q2[:], q_ps[:], AFT.Identity, bias=bq_sb[:]
                        )
                    else:
                        nc.scalar.activation(q2[:], q_ps[:], AFT.Copy)
                    qk = sb.tile([128, A], dt.bfloat16, tag="qk")
                    nc.vector.tensor_tensor(qk[:], q2[:], k_ps[:], AOT.mult)
                    # s[atom, h] per tile via PE reduction over hc partitions
                    # (shares PSUM slots with q_ps, which is dead by now)
                    s_ps = pq.tile([128, 4 * B], dt.float32, tag="qp")
                    for b in range(B):
                        nc.tensor.matmul(
                            s_ps[:, 4 * b : 4 * b + 4],
                            qk[:, 128 * b : 128 * (b + 1)],
                            ind_sb[:],
                            start=True, stop=True,
                        )
                    # atom-major v
                    v_ps = pv.tile([128, A], dt.float32, tag="vp")
                    for b in range(B):
                        nc.tensor.matmul(
                            v_ps[:, 128 * b : 128 * (b + 1)],
                            kv_g[:, off + 128 * b : off + 128 * (b + 1)],
                            wv_sb[:],
                            start=True, stop=True,
                        )
                    # fused rhs tile: per tile 132 cols = [w (128) | e (4)]
                    # e = exp(s) written straight into the e columns by ACT
                    we = sb.tile([128, B * 132], dt.bfloat16, tag="we")
                    we3 = we[:].rearrange("p (b f) -> p b f", b=B, f=132)
                    nc.scalar.activation(
                        we3[:, :, 128:132],
                        s_ps[:].rearrange("p (b h) -> p b h", b=B, h=H),
                        AFT.Exp,
                    )
                    w_view = we3[:, :, 0:128].rearrange(
                        "p b (h c) -> p b h c", h=H, c=C
                    )
                    v_view = v_ps[:].rearrange("p (b h c) -> p b h c", b=B, h=H, c=C)
                    e_view = (
                        we3[:, :, 128:132]
                        .unsqueeze(-1)
                        .broadcast_to((128, B, H, C))
                    )
                    nc.vector.tensor_tensor(w_view, v_view, e_view, AOT.mult)
                    if debug_dumps and g == 0 and b0 == 0:
                        nc.sync.dma_start(dbg["dbg_q2"][:], q2[:])
                        nc.sync.dma_start(dbg["dbg_qk"][:], qk[:])
                        nc.sync.dma_start(dbg["dbg_a16"][:], a16[:, 0:512])
                    # one fused segment-sum matmul per tile: [num | den]
                    for b in range(B):
                        t = b0 + b
                        nc.tensor.matmul(
                            grp_ps[:],
                            a16[:, 128 * t : 128 * (t + 1)],
                            we[:, 132 * b : 132 * (b + 1)],
                            start=(t == 0), stop=(t == cap_tiles - 1),
                        )

                # ---- group tail: normalize + gate into y_all, defer the rest ----
                if debug_dumps and g == 0:
                    grp_cp = sb.tile([128, 132], dt.float32, tag="grpcp")
                    nc.vector.tensor_copy(grp_cp[:], grp_ps[:])
                    nc.sync.dma_start(dbg["dbg_grp"][:], grp_cp[:])
                r32 = sb.tile([128, H], dt.float32, tag="r32")
                nc.vector.reciprocal(r32[:], grp_ps[:, 128:132])
                y0 = sb.tile([128, HC], dt.bfloat16, tag="y0")
                num_view = grp_ps[:, 0:128].rearrange("p (h c) -> p h c", h=H, c=C)
                y0_view = y0[:].rearrange("p (h c) -> p h c", h=H, c=C)
                r_view = r32[:].unsqueeze(-1).broadcast_to((128, H, C))
                nc.vector.tensor_tensor(y0_view, num_view, r_view, AOT.mult)
                nc.vector.tensor_tensor(
                    y_all[:, g * TPG : (g + 1) * TPG],
                    y0[:],
                    gate_all[:, g * TPG : (g + 1) * TPG],
                    AOT.mult,
                )
                if debug_dumps and g == 0:
                    nc.sync.dma_start(dbg["dbg_y0"][:], y0[:])
                    nc.sync.dma_start(dbg["dbg_gate"][:], gate_all[:, 0:TPG])

            # ---- phase 2: transpose + output projection for all groups ----
            for g in range(GPC):
                yT_ps = pk.tile([128, 128], dt.bfloat16, tag="kp")
                nc.tensor.transpose(
                    yT_ps[:], y_all[:, g * TPG : (g + 1) * TPG], ident_sb[:]
                )
                yT16 = sb.tile([128, 128], dt.bfloat16, tag="yT")
                nc.scalar.activation(yT16[:], yT_ps[:], AFT.Copy)
                f_ps = pv.tile([128, C_Q], dt.float32, tag="vp")
                nc.tensor.matmul(
                    f_ps[:], yT16[:], wo_sb[:], start=True, stop=True
                )
                o32 = outp.tile([128, C_Q], dt.float32, tag="o")
                nc.scalar.activation(o32[:], f_ps[:], AFT.Copy)
                nc.sync.dma_start(out_d[g * TPG : (g + 1) * TPG, :], o32[:])

    nc.compile()
    _BUILD_CACHE[key] = nc
    return nc


def _install_ntff_shim():
    """The agent image's `antenv` lacks `axon_hooks`; recreate it and install
    the ctypes NTFF profile hook the way trn_agent_boot would."""
    import types

    import antenv

    if "antenv.axon_hooks" in sys.modules:
        return
    mod = types.ModuleType("antenv.axon_hooks")
    holder = [None]
    mod.set_axon_ntff_profile_hook = lambda h: holder.__setitem__(0, h)
    mod.get_axon_ntff_profile_hook = lambda: holder[0]
    sys.modules["antenv.axon_hooks"] = mod
    antenv.axon_hooks = mod
    try:
        sys.path.insert(0, "/root/.axon_site")
        from trn_agent_boot.trn_boot import _ntff_profile_via_ctypes

        hook = _ntff_profile_via_ctypes("/opt/axon/libaxon_pjrt.so")
        mod.set_axon_ntff_profile_hook(hook)
    except Exception as e:  # degrade to no tracing
        print(f"ntff shim install failed: {e}")


def kernel(q_x, kv_x, atom_to_token_idx, Wq, bq, Wk, Wv, Wg, Wo, bo):
    global LAST_RESULTS
    from concourse.bass_utils import run_bass_kernel_spmd

    q_x = np.asarray(q_x, np.float32)
    kv_x = np.asarray(kv_x, np.float32)
    Wq = np.asarray(Wq, np.float32)
    bq = np.asarray(bq, np.float32)
    Wk = np.asarray(Wk, np.float32)
    Wv = np.asarray(Wv, np.float32)
    Wg = np.asarray(Wg, np.float32)
    Wo = np.asarray(Wo, np.float32)
    bo = np.asarray(bo, np.float32)

    sh = _host_shard(atom_to_token_idx)
    cap_tiles = sh["cap_tiles"]
    cap_atoms = sh["cap_atoms"]
    perm, dest, slots = sh["perm"], sh["dest"], sh["slots"]
    tok_grid = sh["tok_grid"]

    # padded, permuted, bf16 inputs
    tot = GROUPS * cap_atoms
    Xq = np.zeros((tot, 128), BF16)
    Xq[dest] = q_x[perm].astype(BF16)
    Xkv = np.zeros((tot, 128), BF16)
    Xkv[dest] = kv_x[perm].astype(BF16)
    Afull = np.zeros((tot, TPG), BF16)
    Afull[dest, slots] = 1

    wq_h = (Wq * INV_SQRT_C).astype(BF16)
    wk_h = (Wk * INV_SQRT_C).astype(BF16)
    wv_h = Wv.astype(BF16)
    wg_h = Wg.astype(BF16)
    wo_h = Wo.astype(BF16)
    bq_h = (bq * INV_SQRT_C).astype(np.float32).reshape(128, 1)
    ind_h = np.zeros((HC, H), BF16)
    for h in range(H):
        ind_h[h * C : (h + 1) * C, h] = 1
    ident_h = np.eye(128, dtype=BF16)

    apc = GPC * cap_atoms
    in_maps = []
    for c in range(NCORES):
        rows = slice(c * apc, (c + 1) * apc)
        qxT = np.ascontiguousarray(Xq[rows].T)
        kvT = np.ascontiguousarray(Xkv[rows].T)
        aT = np.ascontiguousarray(
            Afull[rows]
            .reshape(GPC * cap_tiles, 128, TPG)
            .transpose(1, 0, 2)
            .reshape(128, apc)
        )
        tok_core = tok_grid[c * GPC : (c + 1) * GPC].reshape(GPC * TPG)
        qxoT = np.ascontiguousarray(q_x[tok_core].T.astype(BF16))
        in_maps.append(
            dict(
                qxT=qxT, kvT=kvT, aT=aT, qxoT=qxoT,
                wq=wq_h, wk=wk_h, wv=wv_h, wg=wg_h, wo=wo_h,
                ind=ind_h, ident=ident_h, bqv=bq_h,
            )
        )

    nc = _build_nc(
        cap_tiles,
        debug_dumps=os.environ.get("KERNEL_DEBUG_DUMPS", "0") == "1",
        has_bq=bool(np.any(bq != 0)),
    )
    trace = os.environ.get("KERNEL_TRACE", "0") == "1"
    if trace:
        _install_ntff_shim()
    res = run_bass_kernel_spmd(
        nc, in_maps, list(range(NCORES)), trace=trace,
        tmpdir=os.environ.get("KERNEL_TRACE_DIR") or None,
    )
    LAST_RESULTS = res

    out_full = np.broadcast_to(bo, (N, C_Q)).astype(np.float32).copy()
    for c in range(NCORES):
        tok_core = tok_grid[c * GPC : (c + 1) * GPC].reshape(GPC * TPG)
        out_full[tok_core] = res.results[c]["out"] + bo
    empty = np.where(sh["counts"] == 0)[0]
    if empty.size:
        out_full[empty] = bo
    return out_full


# revision 51
# speedup vs baseline: 2.7731x; 1.0025x over previous
"""
Trainium2 Bass kernel for nn_Local_Attention (segment-softmax attention over
atoms grouped into tokens).

Algorithm notes (reference semantics):
  q = (q_x @ Wq + bq) / sqrt(C)            [N, H*C]
  k = kv_x @ Wk ; v = kv_x @ Wv            [N, H*C]
  s[i,h] = sum_c q[i,h,c] k[i,h,c] / sqrt(C)
  alpha  = softmax of s over atoms sharing a token (segment softmax)
  out[t] = sum_{i in t} alpha[i] * v[i]    (only rows t < NUM_TOKENS nonzero)
  result = (out * sigmoid(q_x @ Wg)) @ Wo + bo

Key simplifications used here:
  * Scores are tiny (|s| < ~0.1), so the segment-max subtraction is skipped:
    alpha = e / segsum(e), e = exp(s). Numerator and denominator are both
    segment *sums*, and the division happens at token level:
    out[t] = segsum(e*v)[t] / segsum(e)[t].
  * Rows >= NUM_TOKENS of the result equal bo (segment sum there is zero), so
    only the first NUM_TOKENS rows are computed on device.
  * Segment sums are computed as one-hot matmuls: the host sorts atoms by
    token, packs 128 tokens per "group" (LPT-balanced), pads each group's
    atom list to a fixed tile capacity, and each 128-atom tile contributes
    via a [atom, token-slot] one-hot built on device from per-atom slot ids.

Sharding: 128 groups of 128 tokens each; 16 groups per core on 8 cores.
Projection weights are replicated.
"""

import math
import os
import sys

import numpy as np

sys.path.insert(0, "/opt/trn_rl_repo")

import ml_dtypes

BF16 = ml_dtypes.bfloat16

N = 262144
C_Q = 128
C_KV = 128
H = 4
C = 32
HC = H * C  # 128
NUM_TOKENS = 16384
NCORES = 8
GROUPS = 128          # token groups overall
TPG = 128             # tokens per group
GPC = GROUPS // NCORES  # groups per core = 16
INV_SQRT_C = 1.0 / math.sqrt(C)
PAD_SLOT = 255        # slot id for padding atoms (matches no token slot)

_BUILD_CACHE = {}
LAST_RESULTS = None  # stash of the last BassKernelResults for test harness


def _host_shard(atom_to_token_idx):
    """Assign tokens to 128 LPT-balanced groups of 128 tokens, sort atoms by
    (group, token), and compute the padded layout.

    Returns dict with permutation, destination indices, per-atom slot ids,
    token grid, and cap_tiles."""
    idx = np.asarray(atom_to_token_idx).astype(np.int64)
    counts = np.bincount(idx, minlength=NUM_TOKENS)

    # snake-deal tokens (sorted by size desc) into GROUPS groups
    order_tok = np.argsort(-counts, kind="stable")
    rounds = order_tok.reshape(NUM_TOKENS // GROUPS, GROUPS).copy()
    rounds[1::2] = rounds[1::2, ::-1]
    grp_of_tok = np.empty(NUM_TOKENS, np.int64)
    slot_of_tok = np.empty(NUM_TOKENS, np.int64)
    grp_of_tok[rounds] = np.broadcast_to(
        np.arange(GROUPS)[None, :], rounds.shape
    )
    slot_of_tok[rounds] = np.broadcast_to(
        np.arange(rounds.shape[0])[:, None], rounds.shape
    )
    # token id at (group, slot)
    tok_grid = np.empty((GROUPS, TPG), np.int64)
    tok_grid[grp_of_tok, slot_of_tok] = np.arange(NUM_TOKENS)

    loads = counts[tok_grid].sum(axis=1)  # atoms per group
    cap_tiles = max(1, int(math.ceil(loads.max() / 128.0)))
    cap_atoms = cap_tiles * 128

    # atoms sorted by (group, token id)
    key = grp_of_tok[idx] * NUM_TOKENS + idx
    perm = np.argsort(key, kind="stable")
    gidx = grp_of_tok[idx[perm]]           # nondecreasing group per atom
    group_start = np.searchsorted(gidx, np.arange(GROUPS))
    rank = np.arange(N) - group_start[gidx]
    dest = gidx * cap_atoms + rank         # position in padded atom array
    slots = slot_of_tok[idx[perm]]         # token slot of each (permuted) atom

    return dict(
        perm=perm,
        dest=dest,
        slots=slots,
        tok_grid=tok_grid,
        counts=counts,
        cap_tiles=cap_tiles,
        cap_atoms=cap_atoms,
    )


def _build_nc(cap_tiles, debug_dumps=False, has_bq=False):
    """Build + schedule the SPMD Bass program for a given per-group tile
    capacity. Cached per cap_tiles."""
    key = (cap_tiles, debug_dumps, has_bq)
    if key in _BUILD_CACHE:
        return _BUILD_CACHE[key]

    import concourse.bass as bass
    import concourse.tile as tile
    from concourse import bacc, mybir

    dt = mybir.dt
    AOT = mybir.AluOpType
    AFT = mybir.ActivationFunctionType

    cap_atoms = cap_tiles * 128
    atoms_pc = GPC * cap_atoms         # padded atoms per core
    tiles_pc = GPC * cap_tiles

    nc = bacc.Bacc(
        "TRN2", target_bir_lowering=False, debug=False, num_devices=NCORES
    )

    qxT_d = nc.dram_tensor("qxT", [128, atoms_pc], dt.bfloat16, kind="ExternalInput")
    kvT_d = nc.dram_tensor("kvT", [128, atoms_pc], dt.bfloat16, kind="ExternalInput")
    qxoT_d = nc.dram_tensor("qxoT", [128, GPC * TPG], dt.bfloat16, kind="ExternalInput")
    wq_d = nc.dram_tensor("wq", [128, HC], dt.bfloat16, kind="ExternalInput")
    wk_d = nc.dram_tensor("wk", [128, HC], dt.bfloat16, kind="ExternalInput")
    wv_d = nc.dram_tensor("wv", [128, HC], dt.bfloat16, kind="ExternalInput")
    wg_d = nc.dram_tensor("wg", [128, HC], dt.bfloat16, kind="ExternalInput")
    wo_d = nc.dram_tensor("wo", [HC, C_Q], dt.bfloat16, kind="ExternalInput")
    ind_d = nc.dram_tensor("ind", [HC, H], dt.bfloat16, kind="ExternalInput")
    ident_d = nc.dram_tensor("ident", [128, 128], dt.bfloat16, kind="ExternalInput")
    a_d = nc.dram_tensor("aT", [128, atoms_pc], dt.bfloat16, kind="ExternalInput")
    bq_d = nc.dram_tensor("bqv", [128, 1], dt.float32, kind="ExternalInput")
    out_d = nc.dram_tensor("out", [GPC * TPG, C_Q], dt.float32, kind="ExternalOutput")
    dbg = {}
    if debug_dumps:
        for nm, shp, dty in [
            ("dbg_q2", [128, 512], dt.bfloat16),
            ("dbg_qk", [128, 512], dt.bfloat16),
            ("dbg_a16", [128, 512], dt.bfloat16),
            ("dbg_grp", [128, 132], dt.float32),
            ("dbg_y0", [128, 128], dt.bfloat16),
            ("dbg_gate", [128, 128], dt.bfloat16),
            ("dbg_yT", [128, 128], dt.bfloat16),
        ]:
            dbg[nm] = nc.dram_tensor(nm, shp, dty, kind="ExternalOutput")

    # batches of up to 4 tiles within each group
    batches = []
    b0 = 0
    while b0 < cap_tiles:
        B = min(4, cap_tiles - b0)
        batches.append((b0, B))
        b0 += B

    with tile.TileContext(nc) as tc:
        with (
            tc.tile_pool(name="const", bufs=1) as cpool,
            tc.tile_pool(name="inp", bufs=3) as inp,
            tc.tile_pool(name="sb", bufs=3) as sb,
            tc.tile_pool(name="outp", bufs=2) as outp,
            tc.tile_pool(name="pgrp", bufs=1, space=bass.MemorySpace.PSUM) as pgrp,
            tc.tile_pool(name="pq", bufs=3, space=bass.MemorySpace.PSUM) as pq,
            tc.tile_pool(name="pk", bufs=2, space=bass.MemorySpace.PSUM) as pk,
            tc.tile_pool(name="pv", bufs=2, space=bass.MemorySpace.PSUM) as pv,
        ):
            wq_sb = cpool.tile_from(wq_d[:])
            wk_sb = cpool.tile_from(wk_d[:])
            wv_sb = cpool.tile_from(wv_d[:])
            wg_sb = cpool.tile_from(wg_d[:])
            wo_sb = cpool.tile_from(wo_d[:])
            ind_sb = cpool.tile_from(ind_d[:])
            ident_sb = cpool.tile_from(ident_d[:])
            bq_sb = cpool.tile_from(bq_d[:])
            qxoT_sb = cpool.tile_from(qxoT_d[:])

            # gate pre-pass: project + sigmoid all 16 groups' tokens up front
            # (keeps ACT's LUT on Exp for the whole main loop, and the dense
            # matmul burst warms the PE clock gate)
            gate_all = cpool.tile([128, GPC * TPG], dt.bfloat16)
            y_all = cpool.tile([128, GPC * TPG], dt.bfloat16)
            for g in range(GPC):
                g_ps = pq.tile([128, HC], dt.float32, tag="qp")
                nc.tensor.matmul(
                    g_ps[:], qxoT_sb[:, g * TPG : (g + 1) * TPG], wg_sb[:],
                    start=True, stop=True,
                )
                nc.scalar.activation(
                    gate_all[:, g * TPG : (g + 1) * TPG], g_ps[:], AFT.Sigmoid
                )

            for g in range(GPC):
                qx_g = inp.tile([128, cap_atoms], dt.bfloat16, tag="qx")
                nc.sync.dma_start(
                    qx_g[:], qxT_d[:, g * cap_atoms : (g + 1) * cap_atoms]
                )
                kv_g = inp.tile([128, cap_atoms], dt.bfloat16, tag="kv")
                nc.sync.dma_start(
                    kv_g[:], kvT_d[:, g * cap_atoms : (g + 1) * cap_atoms]
                )

                grp_ps = pgrp.tile([128, 132], dt.float32, tag="grp")

                # one-hot [atom, token-slot], host-precomputed, via DMA
                a16 = inp.tile([128, cap_atoms], dt.bfloat16, tag="a")
                nc.sync.dma_start(
                    a16[:], a_d[:, g * cap_atoms : (g + 1) * cap_atoms]
                )

                for (b0, B) in batches:
                    A = B * 128
                    off = b0 * 128
                    # feature-major q, k for the score chain
                    q_ps = pq.tile([128, A], dt.float32, tag="qp")
                    nc.tensor.matmul(
                        q_ps[:], wq_sb[:], qx_g[:, off : off + A],
                        start=True, stop=True,
                    )
                    k_ps = pk.tile([128, A], dt.float32, tag="kp")
                    nc.tensor.matmul(
                        k_ps[:], wk_sb[:], kv_g[:, off : off + A],
                        start=True, stop=True,
                    )
                    # q2 = q + bq (per-partition bias) on ACT, PSUM -> SBUF;
                    # then qk = q2 * k on DVE (only one PSUM operand allowed)
                    q2 = sb.tile([128, A], dt.bfloat16, tag="q2")
                    if has_bq:
                        nc.scalar.activation(
                            q2[:], q_ps[:], AFT.Identity, bias=bq_sb[:]
                        )
                    else:
                        nc.scalar.activation(q2[:], q_ps[:], AFT.Copy)
                    qk = sb.tile([128, A], dt.bfloat16, tag="qk")
                    nc.vector.tensor_tensor(qk[:], q2[:], k_ps[:], AOT.mult)
                    # s[atom, h] per tile via PE reduction over hc partitions
                    # (shares PSUM slots with q_ps, which is dead by now)
                    s_ps = pq.tile([128, 4 * B], dt.float32, tag="qp")
                    for b in range(B):
                        nc.tensor.matmul(
                            s_ps[:, 4 * b : 4 * b + 4],
                            qk[:, 128 * b : 128 * (b + 1)],
                            ind_sb[:],
                            start=True, stop=True,
                        )
                    # atom-major v
                    v_ps = pv.tile([128, A], dt.float32, tag="vp")
                    for b in range(B):
                        nc.tensor.matmul(
                            v_ps[:, 128 * b : 128 * (b + 1)],
                            kv_g[:, off + 128 * b : off + 128 * (b + 1)],
                            wv_sb[:],
                            start=True, stop=True,
                        )
                    # fused rhs tile: per tile 132 cols = [w (128) | e (4)]
                    # e = exp(s) written straight into the e columns by ACT
                    we = sb.tile([128, B * 132], dt.bfloat16, tag="we")
                    we3 = we[:].rearrange("p (b f) -> p b f", b=B, f=132)
                    nc.scalar.activation(
                        we3[:, :, 128:132],
                        s_ps[:].rearrange("p (b h) -> p b h", b=B, h=H),
                        AFT.Exp,
                    )
                    w_view = we3[:, :, 0:128].rearrange(
                        "p b (h c) -> p b h c", h=H, c=C
                    )
                    v_view = v_ps[:].rearrange("p (b h c) -> p b h c", b=B, h=H, c=C)
                    e_view = (
                        we3[:, :, 128:132]
                        .unsqueeze(-1)
                        .broadcast_to((128, B, H, C))
                    )
                    nc.vector.tensor_tensor(w_view, v_view, e_view, AOT.mult)
                    if debug_dumps and g == 0 and b0 == 0:
                        nc.sync.dma_start(dbg["dbg_q2"][:], q2[:])
                        nc.sync.dma_start(dbg["dbg_qk"][:], qk[:])
                        nc.sync.dma_start(dbg["dbg_a16"][:], a16[:, 0:512])
                    # one fused segment-sum matmul per tile: [num | den]
                    for b in range(B):
                        t = b0 + b
                        nc.tensor.matmul(
                            grp_ps[:],
                            a16[:, 128 * t : 128 * (t + 1)],
                            we[:, 132 * b : 132 * (b + 1)],
                            start=(t == 0), stop=(t == cap_tiles - 1),
                        )

                # ---- group tail: normalize + gate into y_all, defer the rest ----
                if debug_dumps and g == 0:
                    grp_cp = sb.tile([128, 132], dt.float32, tag="grpcp")
                    nc.vector.tensor_copy(grp_cp[:], grp_ps[:])
                    nc.sync.dma_start(dbg["dbg_grp"][:], grp_cp[:])
                r32 = sb.tile([128, H], dt.float32, tag="r32")
                nc.vector.reciprocal(r32[:], grp_ps[:, 128:132])
                y0 = sb.tile([128, HC], dt.bfloat16, tag="y0")
                num_view = grp_ps[:, 0:128].rearrange("p (h c) -> p h c", h=H, c=C)
                y0_view = y0[:].rearrange("p (h c) -> p h c", h=H, c=C)
                r_view = r32[:].unsqueeze(-1).broadcast_to((128, H, C))
                nc.vector.tensor_tensor(y0_view, num_view, r_view, AOT.mult)
                nc.vector.tensor_tensor(
                    y_all[:, g * TPG : (g + 1) * TPG],
                    y0[:],
                    gate_all[:, g * TPG : (g + 1) * TPG],
                    AOT.mult,
                )
                if debug_dumps and g == 0:
                    nc.sync.dma_start(dbg["dbg_y0"][:], y0[:])
                    nc.sync.dma_start(dbg["dbg_gate"][:], gate_all[:, 0:TPG])

            # ---- phase 2: transpose + output projection for all groups ----
            for g in range(GPC):
                yT_ps = pk.tile([128, 128], dt.bfloat16, tag="kp")
                nc.tensor.transpose(
                    yT_ps[:], y_all[:, g * TPG : (g + 1) * TPG], ident_sb[:]
                )
                yT16 = sb.tile([128, 128], dt.bfloat16, tag="yT")
                nc.scalar.activation(yT16[:], yT_ps[:], AFT.Copy)
                f_ps = pv.tile([128, C_Q], dt.float32, tag="vp")
                nc.tensor.matmul(
                    f_ps[:], yT16[:], wo_sb[:], start=True, stop=True
                )
                o32 = outp.tile([128, C_Q], dt.float32, tag="o")
                nc.scalar.activation(o32[:], f_ps[:], AFT.Copy)
                nc.sync.dma_start(out_d[g * TPG : (g + 1) * TPG, :], o32[:])

    nc.compile()
    _BUILD_CACHE[key] = nc
    return nc


def _install_ntff_shim():
    """The agent image's `antenv` lacks `axon_hooks`; recreate it and install
    the ctypes NTFF profile hook the way trn_agent_boot would."""
    import types

    import antenv

    if "antenv.axon_hooks" in sys.modules:
        return
    mod = types.ModuleType("antenv.axon_hooks")
    holder = [None]
    mod.set_axon_ntff_profile_hook = lambda h: holder.__setitem__(0, h)
    mod.get_axon_ntff_profile_hook = lambda: holder[0]
    sys.modules["antenv.axon_hooks"] = mod
    antenv.axon_hooks = mod
    try:
        sys.path.insert(0, "/root/.axon_site")
        from trn_agent_boot.trn_boot import _ntff_profile_via_ctypes

        hook = _ntff_profile_via_ctypes("/opt/axon/libaxon_pjrt.so")
        mod.set_axon_ntff_profile_hook(hook)
    except Exception as e:  # degrade to no tracing
        print(f"ntff shim install failed: {e}")


def kernel(q_x, kv_x, atom_to_token_idx, Wq, bq, Wk, Wv, Wg, Wo, bo):
    global LAST_RESULTS
    from concourse.bass_utils import run_bass_kernel_spmd

    q_x = np.asarray(q_x, np.float32)
    kv_x = np.asarray(kv_x, np.float32)
    Wq = np.asarray(Wq, np.float32)
    bq = np.asarray(bq, np.float32)
    Wk = np.asarray(Wk, np.float32)
    Wv = np.asarray(Wv, np.float32)
    Wg = np.asarray(Wg, np.float32)
    Wo = np.asarray(Wo, np.float32)
    bo = np.asarray(bo, np.float32)

    sh = _host_shard(atom_to_token_idx)
    cap_tiles = sh["cap_tiles"]
    cap_atoms = sh["cap_atoms"]
    perm, dest, slots = sh["perm"], sh["dest"], sh["slots"]
    tok_grid = sh["tok_grid"]

    # padded, permuted, bf16 inputs
    tot = GROUPS * cap_atoms
    Xq = np.zeros((tot, 128), BF16)
    Xq[dest] = q_x[perm].astype(BF16)
    Xkv = np.zeros((tot, 128), BF16)
    Xkv[dest] = kv_x[perm].astype(BF16)
    Afull = np.zeros((tot, TPG), BF16)
    Afull[dest, slots] = 1

    wq_h = (Wq * INV_SQRT_C).astype(BF16)
    wk_h = (Wk * INV_SQRT_C).astype(BF16)
    wv_h = Wv.astype(BF16)
    wg_h = Wg.astype(BF16)
    wo_h = Wo.astype(BF16)
    bq_h = (bq * INV_SQRT_C).astype(np.float32).reshape(128, 1)
    ind_h = np.zeros((HC, H), BF16)
    for h in range(H):
        ind_h[h * C : (h + 1) * C, h] = 1
    ident_h = np.eye(128, dtype=BF16)

    apc = GPC * cap_atoms
    in_maps = []
    for c in range(NCORES):
        rows = slice(c * apc, (c + 1) * apc)
        qxT = np.ascontiguousarray(Xq[rows].T)
        kvT = np.ascontiguousarray(Xkv[rows].T)
        aT = np.ascontiguousarray(
            Afull[rows]
            .reshape(GPC * cap_tiles, 128, TPG)
            .transpose(1, 0, 2)
            .reshape(128, apc)
        )
        tok_core = tok_grid[c * GPC : (c + 1) * GPC].reshape(GPC * TPG)
        qxoT = np.ascontiguousarray(q_x[tok_core].T.astype(BF16))
        in_maps.append(
            dict(
                qxT=qxT, kvT=kvT, aT=aT, qxoT=qxoT,
                wq=wq_h, wk=wk_h, wv=wv_h, wg=wg_h, wo=wo_h,
                ind=ind_h, ident=ident_h, bqv=bq_h,
            )
        )

    nc = _build_nc(
        cap_tiles,
        debug_dumps=os.environ.get("KERNEL_DEBUG_DUMPS", "0") == "1",
        has_bq=bool(np.any(bq != 0)),
    )
    trace = os.environ.get("KERNEL_TRACE", "0") == "1"
    if trace:
        _install_ntff_shim()
    res = run_bass_kernel_spmd(
        nc, in_maps, list(range(NCORES)), trace=trace,
        tmpdir=os.environ.get("KERNEL_TRACE_DIR") or None,
    )
    LAST_RESULTS = res

    out_full = np.broadcast_to(bo, (N, C_Q)).astype(np.float32).copy()
    for c in range(NCORES):
        tok_core = tok_grid[c * GPC : (c + 1) * GPC].reshape(GPC * TPG)
        out_full[tok_core] = res.results[c]["out"] + bo
    empty = np.where(sh["counts"] == 0)[0]
    if empty.size:
        out_full[empty] = bo
    return out_full


# revision 54
# speedup vs baseline: 3.9821x; 1.4360x over previous
"""
Trainium2 Bass kernel for nn_Local_Attention (segment-softmax attention over
atoms grouped into tokens).

Algorithm notes (reference semantics):
  q = (q_x @ Wq + bq) / sqrt(C)            [N, H*C]
  k = kv_x @ Wk ; v = kv_x @ Wv            [N, H*C]
  s[i,h] = sum_c q[i,h,c] k[i,h,c] / sqrt(C)
  alpha  = softmax of s over atoms sharing a token (segment softmax)
  out[t] = sum_{i in t} alpha[i] * v[i]    (only rows t < NUM_TOKENS nonzero)
  result = (out * sigmoid(q_x @ Wg)) @ Wo + bo

Key simplifications used here:
  * Scores are tiny (|s| < ~0.1), so the segment-max subtraction is skipped:
    alpha = e / segsum(e), e = exp(s). Numerator and denominator are both
    segment *sums*, and the division happens at token level:
    out[t] = segsum(e*v)[t] / segsum(e)[t].
  * Rows >= NUM_TOKENS of the result equal bo (segment sum there is zero), so
    only the first NUM_TOKENS rows are computed on device.
  * Segment sums are computed as one-hot matmuls: the host sorts atoms by
    token, packs 128 tokens per "group" (LPT-balanced), pads each group's
    atom list to a fixed tile capacity, and each 128-atom tile contributes
    via a [atom, token-slot] one-hot built on device from per-atom slot ids.

Sharding: 128 groups of 128 tokens each; 16 groups per core on 8 cores.
Projection weights are replicated.
"""

import math
import os
import sys

import numpy as np

sys.path.insert(0, "/opt/trn_rl_repo")

import ml_dtypes

BF16 = ml_dtypes.bfloat16

N = 262144
C_Q = 128
C_KV = 128
H = 4
C = 32
HC = H * C  # 128
NUM_TOKENS = 16384
NCORES = 8
GROUPS = 128          # token groups overall
TPG = 128             # tokens per group
GPC = GROUPS // NCORES  # groups per core = 16
INV_SQRT_C = 1.0 / math.sqrt(C)
PAD_SLOT = 255        # slot id for padding atoms (matches no token slot)

_BUILD_CACHE = {}
LAST_RESULTS = None  # stash of the last BassKernelResults for test harness


def _host_shard(atom_to_token_idx):
    """Assign tokens to 128 LPT-balanced groups of 128 tokens, sort atoms by
    (group, token), and compute the padded layout.

    Returns dict with permutation, destination indices, per-atom slot ids,
    token grid, and cap_tiles."""
    idx = np.asarray(atom_to_token_idx).astype(np.int64)
    counts = np.bincount(idx, minlength=NUM_TOKENS)

    # snake-deal tokens (sorted by size desc) into GROUPS groups
    order_tok = np.argsort(-counts, kind="stable")
    rounds = order_tok.reshape(NUM_TOKENS // GROUPS, GROUPS).copy()
    rounds[1::2] = rounds[1::2, ::-1]
    grp_of_tok = np.empty(NUM_TOKENS, np.int64)
    slot_of_tok = np.empty(NUM_TOKENS, np.int64)
    grp_of_tok[rounds] = np.broadcast_to(
        np.arange(GROUPS)[None, :], rounds.shape
    )
    slot_of_tok[rounds] = np.broadcast_to(
        np.arange(rounds.shape[0])[:, None], rounds.shape
    )
    # token id at (group, slot)
    tok_grid = np.empty((GROUPS, TPG), np.int64)
    tok_grid[grp_of_tok, slot_of_tok] = np.arange(NUM_TOKENS)

    loads = counts[tok_grid].sum(axis=1)  # atoms per group
    cap_tiles = max(1, int(math.ceil(loads.max() / 128.0)))
    cap_atoms = cap_tiles * 128

    # atoms sorted by (group, token id)
    key = grp_of_tok[idx] * NUM_TOKENS + idx
    perm = np.argsort(key, kind="stable")
    gidx = grp_of_tok[idx[perm]]           # nondecreasing group per atom
    group_start = np.searchsorted(gidx, np.arange(GROUPS))
    rank = np.arange(N) - group_start[gidx]
    dest = gidx * cap_atoms + rank         # position in padded atom array
    slots = slot_of_tok[idx[perm]]         # token slot of each (permuted) atom

    return dict(
        perm=perm,
        dest=dest,
        slots=slots,
        tok_grid=tok_grid,
        counts=counts,
        cap_tiles=cap_tiles,
        cap_atoms=cap_atoms,
    )


def _build_nc(cap_tiles, debug_dumps=False, has_bq=False):
    """Build + schedule the SPMD Bass program for a given per-group tile
    capacity. Cached per cap_tiles."""
    key = (cap_tiles, debug_dumps, has_bq)
    if key in _BUILD_CACHE:
        return _BUILD_CACHE[key]

    import concourse.bass as bass
    import concourse.tile as tile
    from concourse import bacc, mybir

    dt = mybir.dt
    AOT = mybir.AluOpType
    AFT = mybir.ActivationFunctionType

    cap_atoms = cap_tiles * 128
    atoms_pc = GPC * cap_atoms         # padded atoms per core
    tiles_pc = GPC * cap_tiles

    nc = bacc.Bacc(
        "TRN2", target_bir_lowering=False, debug=False, num_devices=NCORES
    )

    qxT_d = nc.dram_tensor("qxT", [128, atoms_pc], dt.bfloat16, kind="ExternalInput")
    kvT_d = nc.dram_tensor("kvT", [128, atoms_pc], dt.bfloat16, kind="ExternalInput")
    qxoT_d = nc.dram_tensor("qxoT", [128, GPC * TPG], dt.bfloat16, kind="ExternalInput")
    wq_d = nc.dram_tensor("wq", [128, HC], dt.bfloat16, kind="ExternalInput")
    wk_d = nc.dram_tensor("wk", [128, HC], dt.bfloat16, kind="ExternalInput")
    wv_d = nc.dram_tensor("wv", [128, HC], dt.bfloat16, kind="ExternalInput")
    wg_d = nc.dram_tensor("wg", [128, HC], dt.bfloat16, kind="ExternalInput")
    wo_d = nc.dram_tensor("wo", [HC, C_Q], dt.bfloat16, kind="ExternalInput")
    ind_d = nc.dram_tensor("ind", [HC, H], dt.bfloat16, kind="ExternalInput")
    ident_d = nc.dram_tensor("ident", [128, 128], dt.bfloat16, kind="ExternalInput")
    a_d = nc.dram_tensor("aT", [128, atoms_pc], dt.bfloat16, kind="ExternalInput")
    bq_d = nc.dram_tensor("bqv", [128, 1], dt.float32, kind="ExternalInput")
    out_d = nc.dram_tensor("out", [GPC * TPG, C_Q], dt.float32, kind="ExternalOutput")
    dbg = {}
    if debug_dumps:
        for nm, shp, dty in [
            ("dbg_q2", [128, 512], dt.bfloat16),
            ("dbg_qk", [128, 512], dt.bfloat16),
            ("dbg_a16", [128, 512], dt.bfloat16),
            ("dbg_grp", [128, 132], dt.float32),
            ("dbg_y0", [128, 128], dt.bfloat16),
            ("dbg_gate", [128, 128], dt.bfloat16),
            ("dbg_yT", [128, 128], dt.bfloat16),
        ]:
            dbg[nm] = nc.dram_tensor(nm, shp, dty, kind="ExternalOutput")

    # batches of up to 3 tiles within each group (3*128 v-cols + 3*4 s-cols
    # share one PSUM bank)
    batches = []
    b0 = 0
    while b0 < cap_tiles:
        B = min(3, cap_tiles - b0)
        batches.append((b0, B))
        b0 += B

    with tile.TileContext(nc) as tc:
        with (
            tc.tile_pool(name="const", bufs=1) as cpool,
            tc.tile_pool(name="inp", bufs=3) as inp,
            tc.tile_pool(name="sb", bufs=3) as sb,
            tc.tile_pool(name="outp", bufs=2) as outp,
            tc.tile_pool(name="pgrp", bufs=1, space=bass.MemorySpace.PSUM) as pgrp,
            tc.tile_pool(name="pq", bufs=3, space=bass.MemorySpace.PSUM) as pq,
            tc.tile_pool(name="pk", bufs=2, space=bass.MemorySpace.PSUM) as pk,
            tc.tile_pool(name="pv", bufs=2, space=bass.MemorySpace.PSUM) as pv,
        ):
            wq_sb = cpool.tile_from(wq_d[:])
            wk_sb = cpool.tile_from(wk_d[:])
            wv_sb = cpool.tile_from(wv_d[:])
            wg_sb = cpool.tile_from(wg_d[:])
            wo_sb = cpool.tile_from(wo_d[:])
            ind_sb = cpool.tile_from(ind_d[:])
            ident_sb = cpool.tile_from(ident_d[:])
            bq_sb = cpool.tile_from(bq_d[:])
            qxoT_sb = cpool.tile_from(qxoT_d[:])

            # gate pre-pass: project + sigmoid all 16 groups' tokens up front
            # (keeps ACT's LUT on Exp for the whole main loop, and the dense
            # matmul burst warms the PE clock gate)
            gate_all = cpool.tile([128, GPC * TPG], dt.bfloat16)
            y_all = cpool.tile([128, GPC * TPG], dt.bfloat16)
            for g in range(GPC):
                g_ps = pq.tile([128, HC], dt.float32, tag="qp")
                nc.tensor.matmul(
                    g_ps[:], qxoT_sb[:, g * TPG : (g + 1) * TPG], wg_sb[:],
                    start=True, stop=True,
                )
                nc.scalar.activation(
                    gate_all[:, g * TPG : (g + 1) * TPG], g_ps[:], AFT.Sigmoid
                )

            for g in range(GPC):
                qx_g = inp.tile([128, cap_atoms], dt.bfloat16, tag="qx")
                nc.sync.dma_start(
                    qx_g[:], qxT_d[:, g * cap_atoms : (g + 1) * cap_atoms]
                )
                kv_g = inp.tile([128, cap_atoms], dt.bfloat16, tag="kv")
                nc.sync.dma_start(
                    kv_g[:], kvT_d[:, g * cap_atoms : (g + 1) * cap_atoms]
                )

                grp_ps = pgrp.tile([128, 132], dt.float32, tag="grp")

                # one-hot [atom, token-slot], host-precomputed, via DMA
                a16 = inp.tile([128, cap_atoms], dt.bfloat16, tag="a")
                nc.sync.dma_start(
                    a16[:], a_d[:, g * cap_atoms : (g + 1) * cap_atoms]
                )

                for (b0, B) in batches:
                    A = B * 128
                    off = b0 * 128
                    # feature-major q, k for the score chain
                    q_ps = pq.tile([128, A], dt.float32, tag="qp")
                    nc.tensor.matmul(
                        q_ps[:], wq_sb[:], qx_g[:, off : off + A],
                        start=True, stop=True,
                    )
                    k_ps = pk.tile([128, A], dt.float32, tag="kp")
                    nc.tensor.matmul(
                        k_ps[:], wk_sb[:], kv_g[:, off : off + A],
                        start=True, stop=True,
                    )
                    # q2 = q + bq (per-partition bias) on ACT, PSUM -> SBUF;
                    # then qk = q2 * k on DVE (only one PSUM operand allowed)
                    q2 = sb.tile([128, A], dt.bfloat16, tag="q2")
                    if has_bq:
                        nc.scalar.activation(
                            q2[:], q_ps[:], AFT.Identity, bias=bq_sb[:]
                        )
                    else:
                        nc.scalar.activation(q2[:], q_ps[:], AFT.Copy)
                    qk = sb.tile([128, A], dt.bfloat16, tag="qk")
                    nc.vector.tensor_tensor(qk[:], q2[:], k_ps[:], AOT.mult)
                    # one PSUM bank per batch holds atom-major v (B*128 cols)
                    # and the per-tile scores s (B*4 cols at offset 384)
                    v_ps = pv.tile([128, 128 * B + 4 * B], dt.float32, tag="vp")
                    s_off = 128 * B
                    for b in range(B):
                        nc.tensor.matmul(
                            v_ps[:, 128 * b : 128 * (b + 1)],
                            kv_g[:, off + 128 * b : off + 128 * (b + 1)],
                            wv_sb[:],
                            start=True, stop=True,
                        )
                    # s[atom, h] per tile via PE reduction over hc partitions
                    for b in range(B):
                        nc.tensor.matmul(
                            v_ps[:, s_off + 4 * b : s_off + 4 * b + 4],
                            qk[:, 128 * b : 128 * (b + 1)],
                            ind_sb[:],
                            start=True, stop=True,
                        )
                    # fused rhs tile: per tile 132 cols = [w (128) | e (4)]
                    # e = exp(s) written straight into the e columns by ACT
                    we = sb.tile([128, B * 132], dt.bfloat16, tag="we")
                    we3 = we[:].rearrange("p (b f) -> p b f", b=B, f=132)
                    nc.scalar.activation(
                        we3[:, :, 128:132],
                        v_ps[:, s_off : s_off + 4 * B].rearrange(
                            "p (b h) -> p b h", b=B, h=H
                        ),
                        AFT.Exp,
                    )
                    w_view = we3[:, :, 0:128].rearrange(
                        "p b (h c) -> p b h c", h=H, c=C
                    )
                    v_view = v_ps[:, 0 : 128 * B].rearrange(
                        "p (b h c) -> p b h c", b=B, h=H, c=C
                    )
                    e_view = (
                        we3[:, :, 128:132]
                        .unsqueeze(-1)
                        .broadcast_to((128, B, H, C))
                    )
                    nc.vector.tensor_tensor(w_view, v_view, e_view, AOT.mult)
                    if debug_dumps and g == 0 and b0 == 0:
                        nc.sync.dma_start(dbg["dbg_q2"][:], q2[:])
                        nc.sync.dma_start(dbg["dbg_qk"][:], qk[:])
                        nc.sync.dma_start(dbg["dbg_a16"][:], a16[:, 0:512])
                    # one fused segment-sum matmul per tile: [num | den]
                    for b in range(B):
                        t = b0 + b
                        nc.tensor.matmul(
                            grp_ps[:],
                            a16[:, 128 * t : 128 * (t + 1)],
                            we[:, 132 * b : 132 * (b + 1)],
                            start=(t == 0), stop=(t == cap_tiles - 1),
                        )

                # ---- group tail: normalize + gate into y_all, defer the rest ----
                if debug_dumps and g == 0:
                    grp_cp = sb.tile([128, 132], dt.float32, tag="grpcp")
                    nc.vector.tensor_copy(grp_cp[:], grp_ps[:])
                    nc.sync.dma_start(dbg["dbg_grp"][:], grp_cp[:])
                r32 = sb.tile([128, H], dt.float32, tag="r32")
                nc.vector.reciprocal(r32[:], grp_ps[:, 128:132])
                y0 = sb.tile([128, HC], dt.bfloat16, tag="y0")
                num_view = grp_ps[:, 0:128].rearrange("p (h c) -> p h c", h=H, c=C)
                y0_view = y0[:].rearrange("p (h c) -> p h c", h=H, c=C)
                r_view = r32[:].unsqueeze(-1).broadcast_to((128, H, C))
                nc.vector.tensor_tensor(y0_view, num_view, r_view, AOT.mult)
                nc.vector.tensor_tensor(
                    y_all[:, g * TPG : (g + 1) * TPG],
                    y0[:],
                    gate_all[:, g * TPG : (g + 1) * TPG],
                    AOT.mult,
                )
                if debug_dumps and g == 0:
                    nc.sync.dma_start(dbg["dbg_y0"][:], y0[:])
                    nc.sync.dma_start(dbg["dbg_gate"][:], gate_all[:, 0:TPG])

            # ---- phase 2: transpose + output projection for all groups ----
            for g in range(GPC):
                yT_ps = pk.tile([128, 128], dt.bfloat16, tag="kp")
                nc.tensor.transpose(
                    yT_ps[:], y_all[:, g * TPG : (g + 1) * TPG], ident_sb[:]
                )
                yT16 = sb.tile([128, 128], dt.bfloat16, tag="yT")
                nc.scalar.activation(yT16[:], yT_ps[:], AFT.Copy)
                f_ps = pv.tile([128, C_Q], dt.float32, tag="vp")
                nc.tensor.matmul(
                    f_ps[:], yT16[:], wo_sb[:], start=True, stop=True
                )
                o32 = outp.tile([128, C_Q], dt.float32, tag="o")
                nc.scalar.activation(o32[:], f_ps[:], AFT.Copy)
                nc.sync.dma_start(out_d[g * TPG : (g + 1) * TPG, :], o32[:])

    nc.compile()
    _BUILD_CACHE[key] = nc
    return nc


def _install_ntff_shim():
    """The agent image's `antenv` lacks `axon_hooks`; recreate it and install
    the ctypes NTFF profile hook the way trn_agent_boot would."""
    import types

    import antenv

    if "antenv.axon_hooks" in sys.modules:
        return
    mod = types.ModuleType("antenv.axon_hooks")
    holder = [None]
    mod.set_axon_ntff_profile_hook = lambda h: holder.__setitem__(0, h)
    mod.get_axon_ntff_profile_hook = lambda: holder[0]
    sys.modules["antenv.axon_hooks"] = mod
    antenv.axon_hooks = mod
    try:
        sys.path.insert(0, "/root/.axon_site")
        from trn_agent_boot.trn_boot import _ntff_profile_via_ctypes

        hook = _ntff_profile_via_ctypes("/opt/axon/libaxon_pjrt.so")
        mod.set_axon_ntff_profile_hook(hook)
    except Exception as e:  # degrade to no tracing
        print(f"ntff shim install failed: {e}")


def kernel(q_x, kv_x, atom_to_token_idx, Wq, bq, Wk, Wv, Wg, Wo, bo):
    global LAST_RESULTS
    from concourse.bass_utils import run_bass_kernel_spmd

    q_x = np.asarray(q_x, np.float32)
    kv_x = np.asarray(kv_x, np.float32)
    Wq = np.asarray(Wq, np.float32)
    bq = np.asarray(bq, np.float32)
    Wk = np.asarray(Wk, np.float32)
    Wv = np.asarray(Wv, np.float32)
    Wg = np.asarray(Wg, np.float32)
    Wo = np.asarray(Wo, np.float32)
    bo = np.asarray(bo, np.float32)

    sh = _host_shard(atom_to_token_idx)
    cap_tiles = sh["cap_tiles"]
    cap_atoms = sh["cap_atoms"]
    perm, dest, slots = sh["perm"], sh["dest"], sh["slots"]
    tok_grid = sh["tok_grid"]

    # padded, permuted, bf16 inputs
    tot = GROUPS * cap_atoms
    Xq = np.zeros((tot, 128), BF16)
    Xq[dest] = q_x[perm].astype(BF16)
    Xkv = np.zeros((tot, 128), BF16)
    Xkv[dest] = kv_x[perm].astype(BF16)
    Afull = np.zeros((tot, TPG), BF16)
    Afull[dest, slots] = 1

    wq_h = (Wq * INV_SQRT_C).astype(BF16)
    wk_h = (Wk * INV_SQRT_C).astype(BF16)
    wv_h = Wv.astype(BF16)
    wg_h = Wg.astype(BF16)
    wo_h = Wo.astype(BF16)
    bq_h = (bq * INV_SQRT_C).astype(np.float32).reshape(128, 1)
    ind_h = np.zeros((HC, H), BF16)
    for h in range(H):
        ind_h[h * C : (h + 1) * C, h] = 1
    ident_h = np.eye(128, dtype=BF16)

    apc = GPC * cap_atoms
    in_maps = []
    for c in range(NCORES):
        rows = slice(c * apc, (c + 1) * apc)
        qxT = np.ascontiguousarray(Xq[rows].T)
        kvT = np.ascontiguousarray(Xkv[rows].T)
        aT = np.ascontiguousarray(
            Afull[rows]
            .reshape(GPC * cap_tiles, 128, TPG)
            .transpose(1, 0, 2)
            .reshape(128, apc)
        )
        tok_core = tok_grid[c * GPC : (c + 1) * GPC].reshape(GPC * TPG)
        qxoT = np.ascontiguousarray(q_x[tok_core].T.astype(BF16))
        in_maps.append(
            dict(
                qxT=qxT, kvT=kvT, aT=aT, qxoT=qxoT,
                wq=wq_h, wk=wk_h, wv=wv_h, wg=wg_h, wo=wo_h,
                ind=ind_h, ident=ident_h, bqv=bq_h,
            )
        )

    nc = _build_nc(
        cap_tiles,
        debug_dumps=os.environ.get("KERNEL_DEBUG_DUMPS", "0") == "1",
        has_bq=bool(np.any(bq != 0)),
    )
    trace = os.environ.get("KERNEL_TRACE", "0") == "1"
    if trace:
        _install_ntff_shim()
    res = run_bass_kernel_spmd(
        nc, in_maps, list(range(NCORES)), trace=trace,
        tmpdir=os.environ.get("KERNEL_TRACE_DIR") or None,
    )
    LAST_RESULTS = res

    out_full = np.broadcast_to(bo, (N, C_Q)).astype(np.float32).copy()
    for c in range(NCORES):
        tok_core = tok_grid[c * GPC : (c + 1) * GPC].reshape(GPC * TPG)
        out_full[tok_core] = res.results[c]["out"] + bo
    empty = np.where(sh["counts"] == 0)[0]
    if empty.size:
        out_full[empty] = bo
    return out_full
